# revision 4
# baseline (speedup 1.0000x reference)
"""Trainium2 Bass kernel v2 for nn_Attention_72541997629647.

Sharding: segment x head-half. Core c = 2*si + hh owns segment si (768 rows,
the 4 segments are 128-aligned so no boundary masks) and heads
[8*hh, 8*hh+8). Each core computes qkv+RoPE for its 8 heads over its 768
rows, block-diagonal attention (which only needs rows inside the segment),
and a proj partial [DIM, 768] contracted over its 640 attention channels
(5 full 128-partition tiles -> no wasted contraction rows). The host sums
the two partials per segment and adds b_proj.

vs v1 (heads-only sharding): per-core DMA drops 36.7MB -> ~11MB (x slice
instead of full x, bf16 I/O everywhere), proj PE cost drops 25.6us -> 16us,
and psum->sbuf copies move to the idle Pool (gpsimd) engine.

All matmuls run in bf16 (1 cycle/row at any size; fp32 psum accumulate).
The softmax denominator path stays f32/f32r.
"""

import os
import sys

for _p in ("/opt/trn_rl_repo", "/root/.axon_site/_ro/trn_rl_repo"):
    if os.path.isdir(_p) and _p not in sys.path:
        sys.path.insert(0, _p)

import numpy as np

import concourse.bacc as bacc
import concourse.bass as bass
import concourse.mybir as mybir
import concourse.tile as tile
from concourse.bass_utils import run_bass_kernel_spmd
from contextlib import ExitStack

S = 3072
DIM = 1280
H = 16
HD = 80
NCORES = 8
SEG = 768            # rows per segment
HPC = 8              # heads per core
NT = SEG // 128      # 6 s-tiles per core
GROUPS = 4           # head groups of 2 per core
VEXT = 97            # v cols: 80 v + 16 pad + ones at 96

F32 = mybir.dt.float32
F32R = mybir.dt.float32r
BF16 = mybir.dt.bfloat16
NPBF16 = mybir.dt.np(BF16)

CANON_SEGS = tuple((SEG * i, SEG * (i + 1)) for i in range(4))

_CACHE: dict = {}


def _segments_from_cu(cu_seqlens: np.ndarray) -> tuple:
    cu = np.asarray(cu_seqlens).astype(np.int64)
    seg = np.searchsorted(cu, np.arange(S), side="right") - 1
    change = np.nonzero(np.diff(seg))[0]
    starts = np.concatenate([[0], change + 1])
    ends = np.concatenate([change + 1, [S]])
    return tuple((int(a), int(b)) for a, b in zip(starts, ends))


def _chan_runs(h: int):
    """(src_row0, dst_ct, dst_row0, n) runs mapping head h's 80 attention
    channels into the 6x128 packed layout at 96-row pitch (16 pad rows per
    head). Runs are 32+32+16 so every src/dst partition base is 32-aligned
    (hardware requires engine AP partition offsets in {0,32,64,96})."""
    def legal(o):
        # engine AP partition rule: size<=32 at any 32-multiple, <=64 at
        # {0,64}, >64 only at 0
        if o == 0:
            return 128
        if o % 64 == 0:
            return 64
        if o % 32 == 0:
            return 32
        return 0
    runs, src = [], 0
    while src < HD:
        gpos = 96 * h + src
        o = gpos % 128
        n = min(legal(o), legal(src % 128), HD - src, 128 - o)
        assert n > 0
        runs.append((src, gpos // 128, o, n))
        src += n
    return runs


def _build(segments, loop_n: int = 1) -> "bacc.Bacc":
    assert segments == CANON_SEGS
    nc = bacc.Bacc("TRN2", target_bir_lowering=False, debug=False,
                   num_devices=NCORES)

    xblk_d = nc.dram_tensor("xblk", [NT, 128, 10, 128], BF16,
                            kind="ExternalInput")
    wqkvb_d = nc.dram_tensor("wqkvb", [GROUPS, 128, 10, 480], BF16,
                             kind="ExternalInput")
    bqkv_d = nc.dram_tensor("bqkv", [1, GROUPS, 480], BF16,
                            kind="ExternalInput")
    cosb_d = nc.dram_tensor("cosb", [128, NT, HD], F32, kind="ExternalInput")
    sinb_d = nc.dram_tensor("sinb", [128, NT, HD], F32, kind="ExternalInput")
    wptb_d = nc.dram_tensor("wptb", [128, 6, DIM], BF16, kind="ExternalInput")
    ident_d = nc.dram_tensor("ident", [128, 128], BF16, kind="ExternalInput")
    ones_d = nc.dram_tensor("onesrow", [1, 128], BF16, kind="ExternalInput")
    onesf_d = nc.dram_tensor("onesf", [1, 128], F32R, kind="ExternalInput")
    out0_d = nc.dram_tensor("outb0", [2, 128, 5, 512], BF16,
                            kind="ExternalOutput")
    out1a_d = nc.dram_tensor("outb1a", [2, 128, 5, 128], BF16,
                             kind="ExternalOutput")
    out1b_d = nc.dram_tensor("outb1b", [2, 128, 5, 128], BF16,
                             kind="ExternalOutput")

    with tile.TileContext(nc) as tc, ExitStack() as ctx:
        if loop_n > 1:
            ctx.enter_context(tc.For_i(0, loop_n, 1))
        per = ctx.enter_context(tc.tile_pool(name="persist", bufs=1))

        ident_sb = per.tile([128, 128], BF16, tag="ident")
        ones_sb = per.tile([1, 128], BF16, tag="ones")
        onesf_sb = per.tile([1, 128], F32R, tag="onesf")
        bqkv_sb = per.tile([1, GROUPS, 480], BF16, tag="bqkv")
        cos_sb = per.tile([128, NT, HD], F32, tag="cos")
        sin_sb = per.tile([128, NT, HD], F32, tag="sin")
        xt_sb = [per.tile([128, 10, 128], BF16, tag=f"xt{t}", name=f"xt{t}")
                 for t in range(NT)]
        wq_sb = [per.tile([128, 10, 480], BF16, tag=f"wq{g}", name=f"wq{g}")
                 for g in range(GROUPS)]
        wp_sb = per.tile([128, 6, DIM], BF16, tag="wp")
        qkTg = [per.tile([HD, 4, SEG], BF16, tag=f"qkT{g}", name=f"qkT{g}")
                for g in range(GROUPS)]
        v_sb = per.tile([128, HPC, NT, VEXT], BF16, tag="v")
        at_sb = per.tile([128, 6, SEG], BF16, tag="at")
        # the 16 pad rows per 96-row head granule are never written by the
        # normalize stage but ARE read by the proj matmul (against zero
        # weights) -- zero the tile so no inf/NaN garbage reaches the PE
        nc.gpsimd.memset(at_sb[:, :, :], 0.0)

        # v pad columns: zeros at 80:96, softmax-denominator ones at 96;
        # memset on the otherwise-idle Pool engine (a broadcast DMA here
        # would emit 6144 tiny descriptors and block the SP queue ~100us)
        nc.gpsimd.memset(v_sb[:, :, :, HD:VEXT - 1], 0.0)
        nc.gpsimd.memset(v_sb[:, :, :, VEXT - 1:VEXT], 1.0)

        # input DMAs: the HWDGE issue slot is a single shared ~625ns/DMA
        # resource, so use few dense DMAs, ordered by first use
        nc.sync.dma_start(out=xt_sb[0], in_=xblk_d[0])
        nc.sync.dma_start(out=wq_sb[0][:, 0:5, :], in_=wqkvb_d[0, :, 0:5, :])
        nc.sync.dma_start(out=wq_sb[0][:, 5:10, :], in_=wqkvb_d[0, :, 5:10, :])
        nc.sync.dma_start(out=xt_sb[1], in_=xblk_d[1])
        nc.sync.dma_start(out=ones_sb, in_=ones_d[:, :])
        nc.sync.dma_start(out=bqkv_sb, in_=bqkv_d[:, :, :])
        nc.sync.dma_start(out=xt_sb[2], in_=xblk_d[2])
        nc.sync.dma_start(out=cos_sb, in_=cosb_d[:, :, :])
        nc.sync.dma_start(out=sin_sb, in_=sinb_d[:, :, :])
        nc.sync.dma_start(out=xt_sb[3], in_=xblk_d[3])
        nc.sync.dma_start(out=ident_sb, in_=ident_d[:, :])
        nc.sync.dma_start(out=xt_sb[4], in_=xblk_d[4])
        nc.sync.dma_start(out=xt_sb[5], in_=xblk_d[5])
        nc.sync.dma_start(out=wq_sb[1], in_=wqkvb_d[1])
        nc.sync.dma_start(out=onesf_sb, in_=onesf_d[:, :])
        nc.sync.dma_start(out=wq_sb[2], in_=wqkvb_d[2])
        nc.sync.dma_start(out=wq_sb[3], in_=wqkvb_d[3])
        nc.sync.dma_start(out=wp_sb, in_=wptb_d[:, :, :])

        qpp = ctx.enter_context(tc.tile_pool(name="qpp", bufs=2, space="PSUM"))
        tpp = ctx.enter_context(tc.tile_pool(name="tpp", bufs=1, space="PSUM"))
        scp = ctx.enter_context(tc.tile_pool(name="scp", bufs=3, space="PSUM"))
        app = ctx.enter_context(tc.tile_pool(name="app", bufs=2, space="PSUM"))
        ropep = ctx.enter_context(tc.tile_pool(name="ropet", bufs=4))
        qkrop = ctx.enter_context(tc.tile_pool(name="qkro", bufs=3))
        expp = ctx.enter_context(tc.tile_pool(name="expp", bufs=8))
        smp = ctx.enter_context(tc.tile_pool(name="smalls", bufs=4))
        outp = ctx.enter_context(tc.tile_pool(name="outp", bufs=2))

        def emit_tp(g, t, ro):
            tp = tpp.tile([HD, 4, 128], BF16, tag="tp", name="tpps")
            for j in range(4):
                nc.tensor.transpose(tp[:, j, :], ro[:, HD * j:HD * (j + 1)],
                                    ident_sb)
            nc.vector.tensor_copy(qkTg[g][:, :, 128 * t:128 * (t + 1)], tp)

        def emit_A(g):
            """qkv + rope + v copy + qk transposes for head group g;
            yields once per s-tile so B work of the previous group can be
            interleaved between tiles.

            The transposes for tile t are deferred until after tile t+1's
            qkv matmuls so the PE never waits on the DVE RoPE chain."""
            pending = None
            for t in range(NT):
                qp = qpp.tile([128, 480], F32, tag="qp", name="qkvps")
                for dp in range(5):
                    nc.tensor.matmul(qp[:, :], lhsT=xt_sb[t][:, dp, :],
                                     rhs=wq_sb[g][:, dp, :],
                                     start=(dp == 0), stop=False)
                yield
                for dp in range(5, 10):
                    nc.tensor.matmul(qp[:, :], lhsT=xt_sb[t][:, dp, :],
                                     rhs=wq_sb[g][:, dp, :],
                                     start=False, stop=False)
                nc.tensor.matmul(qp[:, :], lhsT=ones_sb[:, :],
                                 rhs=bqkv_sb[:, g, :], start=False, stop=True)
                if pending is not None:
                    emit_tp(g, *pending)

                m1 = ropep.tile([128, 320], BF16, tag="m1")
                m2 = ropep.tile([128, 320], BF16, tag="m2")
                qk_h = qp[:, 0:320].rearrange("p (h d) -> p h d", h=4)
                cos_b = cos_sb[:, t:t + 1, :].to_broadcast([128, 4, HD])
                with nc.allow_low_precision("bf16 matmul inputs"):
                    nc.vector.tensor_mul(
                        m1.rearrange("p (h d) -> p h d", h=4), qk_h, cos_b)
                swap = qp[:, 0:320].rearrange(
                    "p (h x d) -> p h x d", h=4, x=2)[:, :, ::-1, :]
                sin_b = sin_sb[:, t:t + 1, :].rearrange(
                    "p t (x d) -> p (t x) d", x=2)[:, None, :, :] \
                    .to_broadcast([128, 4, 2, HD // 2])
                with nc.allow_low_precision("bf16 matmul inputs"):
                    nc.vector.tensor_mul(
                        m2.rearrange("p (h x d) -> p h x d", h=4, x=2),
                        swap, sin_b)
                ro = qkrop.tile([128, 320], BF16, tag="qkro")
                with nc.allow_low_precision("bf16 matmul inputs"):
                    nc.vector.tensor_add(ro, m1, m2)

                with nc.allow_low_precision("bf16 matmul inputs"):
                    nc.scalar.copy(
                        v_sb[:, 2 * g:2 * g + 2, t, 0:HD],
                        qp[:, 320:480].rearrange("p (e d) -> p e d", e=2))
                pending = (t, ro)
                yield
            emit_tp(g, *pending)

        def emit_B(h, qc0, qc1):
            """attention for core-local head h over q columns [qc0, qc1);
            yields between pipeline stages."""
            g, e = h // 2, h % 2
            qT = qkTg[g][:, e]
            kT = qkTg[g][:, 2 + e]
            qna = qc1 - qc0
            ap_ = app.tile([VEXT, 512], F32, tag="ap", name="attps")
            blocks = list(range(NT))
            for g0 in range(0, NT, 4):
                grp = blocks[g0:g0 + 4]
                exs = []
                for j in grp:
                    sc = scp.tile([128, 512], F32, tag="sc", name="scps")
                    nc.tensor.matmul(sc[:, :qna],
                                     lhsT=kT[:, 128 * j:128 * (j + 1)],
                                     rhs=qT[:, qc0:qc1],
                                     start=True, stop=True)
                    ex = expp.tile([128, 512], BF16, tag="expp")
                    nc.scalar.activation(ex[:, :qna], sc[:, :qna],
                                         mybir.ActivationFunctionType.Exp)
                    exs.append(ex)
                yield
                for j, ex in zip(grp, exs):
                    nc.tensor.matmul(ap_[:, :qna], lhsT=v_sb[:, h, j, :],
                                     rhs=ex[:, :qna],
                                     start=(j == 0), stop=(j == NT - 1))
                yield
            den = smp.tile([1, 512], F32R, tag="den", name="den")
            with nc.allow_low_precision("f32r matmul inputs"):
                if qna == 512:
                    nc.scalar.copy(den[:, :qna], ap_[96:97, :qna])
                else:
                    nc.vector.tensor_copy(den[:, :qna], ap_[96:97, :qna])
            yield
            bc = scp.tile([HD, 512], F32, tag="sc", name="bcps")
            nc.tensor.matmul(bc[:, :qna], lhsT=onesf_sb[:, 0:HD],
                             rhs=den[:, :qna], start=True, stop=True)
            rec = smp.tile([HD, 512], F32, tag="rec", name="rec")
            nc.vector.reciprocal(rec[:, :qna], bc[:, :qna])
            for (src, ct, dst, n) in _chan_runs(h):
                with nc.allow_low_precision("bf16 matmul inputs"):
                    nc.vector.tensor_mul(
                        at_sb[dst:dst + n, ct, qc0:qc1],
                        ap_[src:src + n, :qna],
                        rec[src:src + n, :qna])
            yield

        def emit_C(c0, c1, out_d, finer=False):
            """proj for s columns [c0, c1); dense output DMA per half-dim
            chunk (or per m-tile when finer), issue alternating SP/ACT."""
            n = c1 - c0
            for mh in range(2):
                ob = outp.tile([128, 5, n], BF16, tag="outp")
                for mm in range(5):
                    m = 5 * mh + mm
                    pp = qpp.tile([128, 512], F32, tag="qp", name="prps")
                    for ct in range(6):
                        nc.tensor.matmul(
                            pp[:, :n],
                            lhsT=wp_sb[:, ct, 128 * m:128 * (m + 1)],
                            rhs=at_sb[:, ct, c0:c1],
                            start=(ct == 0), stop=(ct == 5))
                    with nc.allow_low_precision("bf16 output"):
                        nc.scalar.copy(ob[:, mm, :], pp[:, :n])
                    if finer:
                        nc.sync.dma_start(
                            out=out_d[mh, :, mm, :], in_=ob[:, mm, :])
                    yield
                if not finer:
                    nc.sync.dma_start(out=out_d[mh], in_=ob)

        def chain(*gens):
            for gg in gens:
                yield from gg

        def zipgen(gen_a, gen_b):
            """Alternate single steps of two independent streams; drains
            both. Used for head pairs so one head's PE stages fill the
            other's exp-latency holes."""
            a_live, b_live = True, True
            while a_live or b_live:
                if a_live:
                    a_live = next(gen_a, StopIteration) is not StopIteration
                if b_live:
                    b_live = next(gen_b, StopIteration) is not StopIteration
                yield

        def interleave(gen_a, gen_b, ratio):
            """Alternate: one step of gen_a, then `ratio` steps of gen_b.
            Drains both."""
            a_live, b_live = True, True
            while a_live or b_live:
                if a_live:
                    a_live = next(gen_a, StopIteration) is not StopIteration
                if b_live:
                    for _ in range(ratio):
                        if next(gen_b, StopIteration) is StopIteration:
                            b_live = False
                            break

        def gen_group_B(g):
            for e in range(2):
                h = 2 * g + e
                yield from emit_B(h, 0, 512)
                yield from emit_B(h, 512, SEG)

        prev_b = None
        for g in range(GROUPS):
            if prev_b is None:
                for _ in emit_A(g):
                    pass
            else:
                interleave(emit_A(g), prev_b, 4)
            if g < GROUPS - 1:
                prev_b = gen_group_B(g)
        # tail: last group's B with C interleaved once its inputs are ready;
        # the last head's second chunk is split so the final serial
        # norm->proj->DMA chain covers only 128 columns
        h0, h1 = 2 * (GROUPS - 1), 2 * (GROUPS - 1) + 1
        for _ in zipgen(emit_B(h0, 0, 512), emit_B(h1, 0, 512)):
            pass
        interleave(chain(emit_B(h0, 512, SEG), emit_B(h1, 512, 640),
                         emit_B(h1, 640, SEG)),
                   emit_C(0, 512, out0_d), 3)
        for _ in emit_C(512, 640, out1a_d):
            pass
        for _ in emit_C(640, SEG, out1b_d):
            pass

    nc.compile()
    return nc


def _prep_inputs(x, cu_seqlens, rotary_pos_emb, w_qkv, b_qkv, w_proj, b_proj):
    """Host-side shard prep. Returns per-core input dicts."""
    scale = np.float32(1.0 / np.sqrt(np.float32(HD)))
    x = np.asarray(x, np.float32)
    w_qkv = np.asarray(w_qkv, np.float32)
    b_qkv = np.asarray(b_qkv, np.float32)
    w_proj = np.asarray(w_proj, np.float32)
    rot = np.asarray(rotary_pos_emb, np.float32)

    cosw = np.concatenate([np.cos(rot), np.cos(rot)], axis=1)
    sinw = np.concatenate([-np.sin(rot), np.sin(rot)], axis=1)

    ident = np.eye(128, dtype=NPBF16)
    onesrow = np.ones((1, 128), dtype=NPBF16)
    onesf = np.ones((1, 128), dtype=np.float32)

    in_maps = []
    for c in range(NCORES):
        si, hh = c // 2, c % 2
        s0 = SEG * si
        heads = list(range(8 * hh, 8 * hh + 8))

        xs = x[s0:s0 + SEG].astype(NPBF16)  # [768, 1280]
        # xblk[t, p, dp, s'] = x[s0+128t+s', 128dp+p]
        xblk = np.ascontiguousarray(
            xs.reshape(NT, 128, 10, 128).transpose(0, 3, 2, 1))

        # w_qkv rows in per-group order [q_a q_b k_a k_b v_a v_b] x 80
        idx = []
        for g in range(GROUPS):
            a, b = heads[2 * g], heads[2 * g + 1]
            for base, hsel in ((0, a), (0, b), (DIM, a), (DIM, b),
                               (2 * DIM, a), (2 * DIM, b)):
                idx.extend(range(base + hsel * HD, base + (hsel + 1) * HD))
        w_c = w_qkv[idx, :].copy()
        b_c = b_qkv[idx].copy()
        for g in range(GROUPS):
            w_c[480 * g:480 * g + 160] *= scale
            b_c[480 * g:480 * g + 160] *= scale
        # wqkvb[g, p, dp, cc] = w_c[480g+cc, 128dp+p] (dense per-group)
        wqkvb = np.ascontiguousarray(
            w_c.T.reshape(10, 128, GROUPS, 480).transpose(2, 1, 0, 3)
        ).astype(NPBF16)
        bqkvb = np.ascontiguousarray(b_c.reshape(1, GROUPS, 480)).astype(NPBF16)

        # rope tables [128, 6, 80] for this segment's rows
        cosb = np.ascontiguousarray(
            cosw[s0:s0 + SEG].reshape(NT, 128, HD).transpose(1, 0, 2))
        sinb = np.ascontiguousarray(
            sinw[s0:s0 + SEG].reshape(NT, 128, HD).transpose(1, 0, 2))

        # wptb[ct, p, m] = w_proj[m, chan(128ct+p)], chan c -> head
        # heads[c//80], dim c%80
        wptb = np.zeros((6, 128, DIM), np.float32)
        wv = wptb.reshape(768, DIM)
        for hl, habs in enumerate(heads):
            wv[96 * hl:96 * hl + HD] = w_proj[:, habs * HD:(habs + 1) * HD].T
        wptb = np.ascontiguousarray(
            wptb.transpose(1, 0, 2)).astype(NPBF16)  # [128, 6, DIM]

        in_maps.append({
            "xblk": xblk,
            "wqkvb": wqkvb,
            "bqkv": bqkvb,
            "cosb": cosb,
            "sinb": sinb,
            "wptb": np.ascontiguousarray(wptb),
            "ident": ident,
            "onesrow": onesrow,
            "onesf": onesf,
        })
    return in_maps


def run(inputs: dict, trace: bool = False):
    segments = _segments_from_cu(inputs["cu_seqlens"])
    if segments != CANON_SEGS:
        from kernel_legacy import run as legacy_run
        return legacy_run(inputs, trace=trace)
    key = (segments, "v2")
    if key not in _CACHE:
        _CACHE[key] = _build(segments)
    nc = _CACHE[key]
    in_maps = _prep_inputs(
        inputs["x"], inputs["cu_seqlens"], inputs["rotary_pos_emb"],
        inputs["w_qkv"], inputs["b_qkv"], inputs["w_proj"], inputs["b_proj"])
    res = run_bass_kernel_spmd(nc, in_maps, core_ids=list(range(NCORES)),
                               trace=trace)
    acc = np.zeros((DIM, S), np.float64)
    for c, r in enumerate(res.results):
        si = c // 2
        part = np.zeros((128, 10, SEG), np.float64)
        for mh in range(2):
            part[:, 5 * mh:5 * (mh + 1), 0:512] = r["outb0"][mh]
            part[:, 5 * mh:5 * (mh + 1), 512:640] = r["outb1a"][mh]
            part[:, 5 * mh:5 * (mh + 1), 640:768] = r["outb1b"][mh]
        # partial[128m+p, s'] = part[p, m, s']
        acc[:, SEG * si:SEG * (si + 1)] += part.transpose(1, 0, 2).reshape(
            DIM, SEG)
    acc += np.asarray(inputs["b_proj"], np.float64)[:, None]
    out = np.ascontiguousarray(acc.T.astype(np.float32))
    return out, res


def kernel(**inputs) -> np.ndarray:
    out, _ = run(inputs, trace=False)
    return out


# revision 5
# speedup vs baseline: 1.0670x; 1.0670x over previous
"""Trainium2 Bass kernel v2 for nn_Attention_72541997629647.

Sharding: segment x head-half. Core c = 2*si + hh owns segment si (768 rows,
the 4 segments are 128-aligned so no boundary masks) and heads
[8*hh, 8*hh+8). Each core computes qkv+RoPE for its 8 heads over its 768
rows, block-diagonal attention (which only needs rows inside the segment),
and a proj partial [DIM, 768] contracted over its 640 attention channels
(5 full 128-partition tiles -> no wasted contraction rows). The host sums
the two partials per segment and adds b_proj.

vs v1 (heads-only sharding): per-core DMA drops 36.7MB -> ~11MB (x slice
instead of full x, bf16 I/O everywhere), proj PE cost drops 25.6us -> 16us,
and psum->sbuf copies move to the idle Pool (gpsimd) engine.

All matmuls run in bf16 (1 cycle/row at any size; fp32 psum accumulate).
The softmax denominator path stays f32/f32r.
"""

import os
import sys

for _p in ("/opt/trn_rl_repo", "/root/.axon_site/_ro/trn_rl_repo"):
    if os.path.isdir(_p) and _p not in sys.path:
        sys.path.insert(0, _p)

import numpy as np

import concourse.bacc as bacc
import concourse.bass as bass
import concourse.mybir as mybir
import concourse.tile as tile
from concourse.bass_utils import run_bass_kernel_spmd
from contextlib import ExitStack

S = 3072
DIM = 1280
H = 16
HD = 80
NCORES = 8
SEG = 768            # rows per segment
HPC = 8              # heads per core
NT = SEG // 128      # 6 s-tiles per core
GROUPS = 4           # head groups of 2 per core
VEXT = 97            # v cols: 80 v + 16 pad + ones at 96

F32 = mybir.dt.float32
F32R = mybir.dt.float32r
BF16 = mybir.dt.bfloat16
NPBF16 = mybir.dt.np(BF16)

CANON_SEGS = tuple((SEG * i, SEG * (i + 1)) for i in range(4))

_CACHE: dict = {}


def _segments_from_cu(cu_seqlens: np.ndarray) -> tuple:
    cu = np.asarray(cu_seqlens).astype(np.int64)
    seg = np.searchsorted(cu, np.arange(S), side="right") - 1
    change = np.nonzero(np.diff(seg))[0]
    starts = np.concatenate([[0], change + 1])
    ends = np.concatenate([change + 1, [S]])
    return tuple((int(a), int(b)) for a, b in zip(starts, ends))


def _chan_runs(h: int):
    """(src_row0, dst_ct, dst_row0, n) runs mapping head h's 80 attention
    channels into the 6x128 packed layout at 96-row pitch (16 pad rows per
    head). Runs are 32+32+16 so every src/dst partition base is 32-aligned
    (hardware requires engine AP partition offsets in {0,32,64,96})."""
    def legal(o):
        # engine AP partition rule: size<=32 at any 32-multiple, <=64 at
        # {0,64}, >64 only at 0
        if o == 0:
            return 128
        if o % 64 == 0:
            return 64
        if o % 32 == 0:
            return 32
        return 0
    runs, src = [], 0
    while src < HD:
        gpos = 96 * h + src
        o = gpos % 128
        n = min(legal(o), legal(src % 128), HD - src, 128 - o)
        assert n > 0
        runs.append((src, gpos // 128, o, n))
        src += n
    return runs


def _build(segments, loop_n: int = 1) -> "bacc.Bacc":
    assert segments == CANON_SEGS
    nc = bacc.Bacc("TRN2", target_bir_lowering=False, debug=False,
                   num_devices=NCORES)

    xblk_d = nc.dram_tensor("xblk", [NT, 128, 10, 128], BF16,
                            kind="ExternalInput")
    wqkvb_d = nc.dram_tensor("wqkvb", [GROUPS, 128, 10, 480], BF16,
                             kind="ExternalInput")
    bqkv_d = nc.dram_tensor("bqkv", [1, GROUPS, 480], BF16,
                            kind="ExternalInput")
    cosb_d = nc.dram_tensor("cosb", [128, NT, HD], F32, kind="ExternalInput")
    sinb_d = nc.dram_tensor("sinb", [128, NT, HD], F32, kind="ExternalInput")
    wptb_d = nc.dram_tensor("wptb", [128, 6, DIM], BF16, kind="ExternalInput")
    ident_d = nc.dram_tensor("ident", [128, 128], BF16, kind="ExternalInput")
    ones_d = nc.dram_tensor("onesrow", [1, 128], BF16, kind="ExternalInput")
    onesf_d = nc.dram_tensor("onesf", [1, 128], F32R, kind="ExternalInput")
    out0_d = nc.dram_tensor("outb0", [2, 128, 5, 512], BF16,
                            kind="ExternalOutput")
    out1a_d = nc.dram_tensor("outb1a", [2, 128, 5, 128], BF16,
                             kind="ExternalOutput")
    out1b_d = nc.dram_tensor("outb1b", [2, 128, 5, 128], BF16,
                             kind="ExternalOutput")

    with tile.TileContext(nc) as tc, ExitStack() as ctx:
        if loop_n > 1:
            ctx.enter_context(tc.For_i(0, loop_n, 1))
        per = ctx.enter_context(tc.tile_pool(name="persist", bufs=1))

        ident_sb = per.tile([128, 128], BF16, tag="ident")
        ones_sb = per.tile([1, 128], BF16, tag="ones")
        onesf_sb = per.tile([1, 128], F32R, tag="onesf")
        bqkv_sb = per.tile([1, GROUPS, 480], BF16, tag="bqkv")
        cos_sb = per.tile([128, NT, HD], F32, tag="cos")
        sin_sb = per.tile([128, NT, HD], F32, tag="sin")
        xt_sb = [per.tile([128, 10, 128], BF16, tag=f"xt{t}", name=f"xt{t}")
                 for t in range(NT)]
        wq_sb = [per.tile([128, 10, 480], BF16, tag=f"wq{g}", name=f"wq{g}")
                 for g in range(GROUPS)]
        wp_sb = per.tile([128, 6, DIM], BF16, tag="wp")
        qkTg = [per.tile([HD, 4, SEG], BF16, tag=f"qkT{g}", name=f"qkT{g}")
                for g in range(GROUPS)]
        v_sb = per.tile([128, HPC, NT, VEXT], BF16, tag="v")
        at_sb = per.tile([128, 6, SEG], BF16, tag="at")
        # the 16 pad rows per 96-row head granule are never written by the
        # normalize stage but ARE read by the proj matmul (against zero
        # weights) -- zero the tile so no inf/NaN garbage reaches the PE
        nc.gpsimd.memset(at_sb[:, :, :], 0.0)

        # v pad columns: zeros at 80:96, softmax-denominator ones at 96;
        # memset on the otherwise-idle Pool engine (a broadcast DMA here
        # would emit 6144 tiny descriptors and block the SP queue ~100us)
        nc.gpsimd.memset(v_sb[:, :, :, HD:VEXT - 1], 0.0)
        nc.gpsimd.memset(v_sb[:, :, :, VEXT - 1:VEXT], 1.0)

        # input DMAs: the HWDGE issue slot is a single shared ~625ns/DMA
        # resource, so use few dense DMAs, ordered by first use
        nc.sync.dma_start(out=xt_sb[0], in_=xblk_d[0])
        nc.sync.dma_start(out=wq_sb[0][:, 0:5, :], in_=wqkvb_d[0, :, 0:5, :])
        nc.sync.dma_start(out=wq_sb[0][:, 5:10, :], in_=wqkvb_d[0, :, 5:10, :])
        nc.sync.dma_start(out=xt_sb[1], in_=xblk_d[1])
        nc.sync.dma_start(out=ones_sb, in_=ones_d[:, :])
        nc.sync.dma_start(out=bqkv_sb, in_=bqkv_d[:, :, :])
        nc.sync.dma_start(out=xt_sb[2], in_=xblk_d[2])
        nc.sync.dma_start(out=cos_sb, in_=cosb_d[:, :, :])
        nc.sync.dma_start(out=sin_sb, in_=sinb_d[:, :, :])
        nc.sync.dma_start(out=xt_sb[3], in_=xblk_d[3])
        nc.sync.dma_start(out=ident_sb, in_=ident_d[:, :])
        nc.sync.dma_start(out=xt_sb[4], in_=xblk_d[4])
        nc.sync.dma_start(out=xt_sb[5], in_=xblk_d[5])
        nc.sync.dma_start(out=wq_sb[1], in_=wqkvb_d[1])
        nc.sync.dma_start(out=onesf_sb, in_=onesf_d[:, :])
        nc.sync.dma_start(out=wq_sb[2], in_=wqkvb_d[2])
        nc.sync.dma_start(out=wq_sb[3], in_=wqkvb_d[3])
        nc.sync.dma_start(out=wp_sb, in_=wptb_d[:, :, :])

        qpp = ctx.enter_context(tc.tile_pool(name="qpp", bufs=2, space="PSUM"))
        tpp = ctx.enter_context(tc.tile_pool(name="tpp", bufs=1, space="PSUM"))
        scp = ctx.enter_context(tc.tile_pool(name="scp", bufs=3, space="PSUM"))
        app = ctx.enter_context(tc.tile_pool(name="app", bufs=2, space="PSUM"))
        ropep = ctx.enter_context(tc.tile_pool(name="ropet", bufs=4))
        qkrop = ctx.enter_context(tc.tile_pool(name="qkro", bufs=4))
        expp = ctx.enter_context(tc.tile_pool(name="expp", bufs=8))
        smp = ctx.enter_context(tc.tile_pool(name="smalls", bufs=4))
        outp = ctx.enter_context(tc.tile_pool(name="outp", bufs=3))

        def emit_tp(g, t, ro):
            tp = tpp.tile([HD, 4, 128], BF16, tag="tp", name="tpps")
            for j in range(4):
                nc.tensor.transpose(tp[:, j, :], ro[:, HD * j:HD * (j + 1)],
                                    ident_sb)
            nc.vector.tensor_copy(qkTg[g][:, :, 128 * t:128 * (t + 1)], tp)

        def emit_A(g):
            """qkv + rope + v copy + qk transposes for head group g;
            yields once per s-tile so B work of the previous group can be
            interleaved between tiles.

            The transposes for tile t are deferred until after tile t+1's
            qkv matmuls so the PE never waits on the DVE RoPE chain."""
            pending = None
            for t in range(NT):
                qp = qpp.tile([128, 480], F32, tag="qp", name="qkvps")
                for dp in range(5):
                    nc.tensor.matmul(qp[:, :], lhsT=xt_sb[t][:, dp, :],
                                     rhs=wq_sb[g][:, dp, :],
                                     start=(dp == 0), stop=False)
                yield
                for dp in range(5, 10):
                    nc.tensor.matmul(qp[:, :], lhsT=xt_sb[t][:, dp, :],
                                     rhs=wq_sb[g][:, dp, :],
                                     start=False, stop=False)
                nc.tensor.matmul(qp[:, :], lhsT=ones_sb[:, :],
                                 rhs=bqkv_sb[:, g, :], start=False, stop=True)
                if pending is not None:
                    emit_tp(g, *pending)

                m1 = ropep.tile([128, 320], BF16, tag="m1")
                m2 = ropep.tile([128, 320], BF16, tag="m2")
                qk_h = qp[:, 0:320].rearrange("p (h d) -> p h d", h=4)
                cos_b = cos_sb[:, t:t + 1, :].to_broadcast([128, 4, HD])
                with nc.allow_low_precision("bf16 matmul inputs"):
                    nc.vector.tensor_mul(
                        m1.rearrange("p (h d) -> p h d", h=4), qk_h, cos_b)
                swap = qp[:, 0:320].rearrange(
                    "p (h x d) -> p h x d", h=4, x=2)[:, :, ::-1, :]
                sin_b = sin_sb[:, t:t + 1, :].rearrange(
                    "p t (x d) -> p (t x) d", x=2)[:, None, :, :] \
                    .to_broadcast([128, 4, 2, HD // 2])
                with nc.allow_low_precision("bf16 matmul inputs"):
                    nc.vector.tensor_mul(
                        m2.rearrange("p (h x d) -> p h x d", h=4, x=2),
                        swap, sin_b)
                ro = qkrop.tile([128, 320], BF16, tag="qkro")
                with nc.allow_low_precision("bf16 matmul inputs"):
                    nc.vector.tensor_add(ro, m1, m2)

                with nc.allow_low_precision("bf16 matmul inputs"):
                    nc.scalar.copy(
                        v_sb[:, 2 * g:2 * g + 2, t, 0:HD],
                        qp[:, 320:480].rearrange("p (e d) -> p e d", e=2))
                pending = (t, ro)
                yield
            emit_tp(g, *pending)

        def emit_B(h, qc0, qc1):
            """attention for core-local head h over q columns [qc0, qc1);
            yields between pipeline stages."""
            g, e = h // 2, h % 2
            qT = qkTg[g][:, e]
            kT = qkTg[g][:, 2 + e]
            qna = qc1 - qc0
            ap_ = app.tile([VEXT, 512], F32, tag="ap", name="attps")
            blocks = list(range(NT))
            for g0 in range(0, NT, 3):
                grp = blocks[g0:g0 + 3]
                exs = []
                for j in grp:
                    sc = scp.tile([128, 512], F32, tag="sc", name="scps")
                    nc.tensor.matmul(sc[:, :qna],
                                     lhsT=kT[:, 128 * j:128 * (j + 1)],
                                     rhs=qT[:, qc0:qc1],
                                     start=True, stop=True)
                    ex = expp.tile([128, 512], BF16, tag="expp")
                    nc.scalar.activation(ex[:, :qna], sc[:, :qna],
                                         mybir.ActivationFunctionType.Exp)
                    exs.append(ex)
                yield
                for j, ex in zip(grp, exs):
                    nc.tensor.matmul(ap_[:, :qna], lhsT=v_sb[:, h, j, :],
                                     rhs=ex[:, :qna],
                                     start=(j == 0), stop=(j == NT - 1))
                yield
            den = smp.tile([1, 512], F32R, tag="den", name="den")
            with nc.allow_low_precision("f32r matmul inputs"):
                if qna == 512:
                    nc.scalar.copy(den[:, :qna], ap_[96:97, :qna])
                else:
                    nc.vector.tensor_copy(den[:, :qna], ap_[96:97, :qna])
            yield
            bc = scp.tile([HD, 512], F32, tag="sc", name="bcps")
            nc.tensor.matmul(bc[:, :qna], lhsT=onesf_sb[:, 0:HD],
                             rhs=den[:, :qna], start=True, stop=True)
            rec = smp.tile([HD, 512], F32, tag="rec", name="rec")
            nc.vector.reciprocal(rec[:, :qna], bc[:, :qna])
            for (src, ct, dst, n) in _chan_runs(h):
                with nc.allow_low_precision("bf16 matmul inputs"):
                    nc.vector.tensor_mul(
                        at_sb[dst:dst + n, ct, qc0:qc1],
                        ap_[src:src + n, :qna],
                        rec[src:src + n, :qna])
            yield

        def emit_C(c0, c1, out_d, finer=False):
            """proj for s columns [c0, c1); dense output DMA per half-dim
            chunk (or per m-tile when finer), issue alternating SP/ACT."""
            n = c1 - c0
            for mh in range(2):
                ob = outp.tile([128, 5, n], BF16, tag="outp")
                for mm in range(5):
                    m = 5 * mh + mm
                    pp = qpp.tile([128, 512], F32, tag="qp", name="prps")
                    for ct in range(6):
                        nc.tensor.matmul(
                            pp[:, :n],
                            lhsT=wp_sb[:, ct, 128 * m:128 * (m + 1)],
                            rhs=at_sb[:, ct, c0:c1],
                            start=(ct == 0), stop=(ct == 5))
                    with nc.allow_low_precision("bf16 output"):
                        nc.scalar.copy(ob[:, mm, :], pp[:, :n])
                    if finer:
                        nc.sync.dma_start(
                            out=out_d[mh, :, mm, :], in_=ob[:, mm, :])
                    yield
                if not finer:
                    nc.sync.dma_start(out=out_d[mh], in_=ob)

        def chain(*gens):
            for gg in gens:
                yield from gg

        def zipgen(gen_a, gen_b):
            """Alternate single steps of two independent streams; drains
            both. Used for head pairs so one head's PE stages fill the
            other's exp-latency holes."""
            a_live, b_live = True, True
            while a_live or b_live:
                if a_live:
                    a_live = next(gen_a, StopIteration) is not StopIteration
                if b_live:
                    b_live = next(gen_b, StopIteration) is not StopIteration
                yield

        def interleave(gen_a, gen_b, ratio):
            """Alternate: one step of gen_a, then `ratio` steps of gen_b.
            Drains both."""
            a_live, b_live = True, True
            while a_live or b_live:
                if a_live:
                    a_live = next(gen_a, StopIteration) is not StopIteration
                if b_live:
                    for _ in range(ratio):
                        if next(gen_b, StopIteration) is StopIteration:
                            b_live = False
                            break

        def gen_group_B(g):
            for e in range(2):
                h = 2 * g + e
                yield from emit_B(h, 0, 512)
                yield from emit_B(h, 512, SEG)

        prev_b = None
        for g in range(GROUPS):
            if prev_b is None:
                for _ in emit_A(g):
                    pass
            else:
                interleave(emit_A(g), prev_b, 4)
            if g < GROUPS - 1:
                prev_b = gen_group_B(g)
        # tail: last group's B with C interleaved once its inputs are ready;
        # the last head's second chunk is split so the final serial
        # norm->proj->DMA chain covers only 128 columns
        h0, h1 = 2 * (GROUPS - 1), 2 * (GROUPS - 1) + 1
        for _ in zipgen(emit_B(h0, 0, 512), emit_B(h1, 0, 512)):
            pass
        interleave(chain(emit_B(h0, 512, SEG), emit_B(h1, 512, 640),
                         emit_B(h1, 640, SEG)),
                   emit_C(0, 512, out0_d), 3)
        for _ in emit_C(512, 640, out1a_d):
            pass
        for _ in emit_C(640, SEG, out1b_d):
            pass

    nc.compile()
    return nc


def _prep_inputs(x, cu_seqlens, rotary_pos_emb, w_qkv, b_qkv, w_proj, b_proj):
    """Host-side shard prep. Returns per-core input dicts."""
    scale = np.float32(1.0 / np.sqrt(np.float32(HD)))
    x = np.asarray(x, np.float32)
    w_qkv = np.asarray(w_qkv, np.float32)
    b_qkv = np.asarray(b_qkv, np.float32)
    w_proj = np.asarray(w_proj, np.float32)
    rot = np.asarray(rotary_pos_emb, np.float32)

    cosw = np.concatenate([np.cos(rot), np.cos(rot)], axis=1)
    sinw = np.concatenate([-np.sin(rot), np.sin(rot)], axis=1)

    ident = np.eye(128, dtype=NPBF16)
    onesrow = np.ones((1, 128), dtype=NPBF16)
    onesf = np.ones((1, 128), dtype=np.float32)

    in_maps = []
    for c in range(NCORES):
        si, hh = c // 2, c % 2
        s0 = SEG * si
        heads = list(range(8 * hh, 8 * hh + 8))

        xs = x[s0:s0 + SEG].astype(NPBF16)  # [768, 1280]
        # xblk[t, p, dp, s'] = x[s0+128t+s', 128dp+p]
        xblk = np.ascontiguousarray(
            xs.reshape(NT, 128, 10, 128).transpose(0, 3, 2, 1))

        # w_qkv rows in per-group order [q_a q_b k_a k_b v_a v_b] x 80
        idx = []
        for g in range(GROUPS):
            a, b = heads[2 * g], heads[2 * g + 1]
            for base, hsel in ((0, a), (0, b), (DIM, a), (DIM, b),
                               (2 * DIM, a), (2 * DIM, b)):
                idx.extend(range(base + hsel * HD, base + (hsel + 1) * HD))
        w_c = w_qkv[idx, :].copy()
        b_c = b_qkv[idx].copy()
        for g in range(GROUPS):
            w_c[480 * g:480 * g + 160] *= scale
            b_c[480 * g:480 * g + 160] *= scale
        # wqkvb[g, p, dp, cc] = w_c[480g+cc, 128dp+p] (dense per-group)
        wqkvb = np.ascontiguousarray(
            w_c.T.reshape(10, 128, GROUPS, 480).transpose(2, 1, 0, 3)
        ).astype(NPBF16)
        bqkvb = np.ascontiguousarray(b_c.reshape(1, GROUPS, 480)).astype(NPBF16)

        # rope tables [128, 6, 80] for this segment's rows
        cosb = np.ascontiguousarray(
            cosw[s0:s0 + SEG].reshape(NT, 128, HD).transpose(1, 0, 2))
        sinb = np.ascontiguousarray(
            sinw[s0:s0 + SEG].reshape(NT, 128, HD).transpose(1, 0, 2))

        # wptb[ct, p, m] = w_proj[m, chan(128ct+p)], chan c -> head
        # heads[c//80], dim c%80
        wptb = np.zeros((6, 128, DIM), np.float32)
        wv = wptb.reshape(768, DIM)
        for hl, habs in enumerate(heads):
            wv[96 * hl:96 * hl + HD] = w_proj[:, habs * HD:(habs + 1) * HD].T
        wptb = np.ascontiguousarray(
            wptb.transpose(1, 0, 2)).astype(NPBF16)  # [128, 6, DIM]

        in_maps.append({
            "xblk": xblk,
            "wqkvb": wqkvb,
            "bqkv": bqkvb,
            "cosb": cosb,
            "sinb": sinb,
            "wptb": np.ascontiguousarray(wptb),
            "ident": ident,
            "onesrow": onesrow,
            "onesf": onesf,
        })
    return in_maps


def run(inputs: dict, trace: bool = False):
    segments = _segments_from_cu(inputs["cu_seqlens"])
    if segments != CANON_SEGS:
        from kernel_legacy import run as legacy_run
        return legacy_run(inputs, trace=trace)
    key = (segments, "v2")
    if key not in _CACHE:
        _CACHE[key] = _build(segments)
    nc = _CACHE[key]
    in_maps = _prep_inputs(
        inputs["x"], inputs["cu_seqlens"], inputs["rotary_pos_emb"],
        inputs["w_qkv"], inputs["b_qkv"], inputs["w_proj"], inputs["b_proj"])
    res = run_bass_kernel_spmd(nc, in_maps, core_ids=list(range(NCORES)),
                               trace=trace)
    acc = np.zeros((DIM, S), np.float64)
    for c, r in enumerate(res.results):
        si = c // 2
        part = np.zeros((128, 10, SEG), np.float64)
        for mh in range(2):
            part[:, 5 * mh:5 * (mh + 1), 0:512] = r["outb0"][mh]
            part[:, 5 * mh:5 * (mh + 1), 512:640] = r["outb1a"][mh]
            part[:, 5 * mh:5 * (mh + 1), 640:768] = r["outb1b"][mh]
        # partial[128m+p, s'] = part[p, m, s']
        acc[:, SEG * si:SEG * (si + 1)] += part.transpose(1, 0, 2).reshape(
            DIM, SEG)
    acc += np.asarray(inputs["b_proj"], np.float64)[:, None]
    out = np.ascontiguousarray(acc.T.astype(np.float32))
    return out, res


def kernel(**inputs) -> np.ndarray:
    out, _ = run(inputs, trace=False)
    return out


# revision 6
# speedup vs baseline: 1.1466x; 1.0746x over previous
"""Trainium2 Bass kernel v2 for nn_Attention_72541997629647.

Sharding: segment x head-half. Core c = 2*si + hh owns segment si (768 rows,
the 4 segments are 128-aligned so no boundary masks) and heads
[8*hh, 8*hh+8). Each core computes qkv+RoPE for its 8 heads over its 768
rows, block-diagonal attention (which only needs rows inside the segment),
and a proj partial [DIM, 768] contracted over its 640 attention channels
(5 full 128-partition tiles -> no wasted contraction rows). The host sums
the two partials per segment and adds b_proj.

Attention channels are packed into 6 contraction tiles at a 96-row pitch
(16 zeroed pad rows per head) because engine AP partition offsets must be
32-aligned; the proj matmul then uses 6 mostly-full 128-partition
contraction tiles instead of 8 sparse 80-row ones.

vs v1 (heads-only sharding): per-core DMA drops 36.7MB -> ~11MB (x slice
instead of full x, bf16 I/O everywhere), proj PE cost drops 25.6us ->
19.2us, and emission is software-pipelined: A(g+1) qkv work interleaves
with B(g) attention so the PE never waits on the ACT exp chain, transposes
are deferred one tile, and the last head's tail is split so the final
serial norm->proj->DMA chain covers only 128 columns.

DMA discipline (the big hardware win): the HWDGE issue slot is a single
~650ns/DMA resource and transfers serialize on the DMA engines, so inputs
are few dense DMAs on one queue in first-use order; v-pad columns are
Pool-engine memsets instead of a broadcast DMA (which would generate 6144
descriptors and block the queue ~100us); outputs are dense per-chunk
buffers.

All matmuls run in bf16 (1 cycle/row at any size; fp32 psum accumulate).
The softmax denominator path stays f32/f32r. Falls back to the embedded
v1 kernel for non-canonical cu_seqlens.
"""

import os
import sys

for _p in ("/opt/trn_rl_repo", "/root/.axon_site/_ro/trn_rl_repo"):
    if os.path.isdir(_p) and _p not in sys.path:
        sys.path.insert(0, _p)

import numpy as np

import concourse.bacc as bacc
import concourse.bass as bass
import concourse.mybir as mybir
import concourse.tile as tile
from concourse.bass_utils import run_bass_kernel_spmd
from contextlib import ExitStack

S = 3072
DIM = 1280
H = 16
HD = 80
NCORES = 8
SEG = 768            # rows per segment
HPC = 8              # heads per core
NT = SEG // 128      # 6 s-tiles per core
GROUPS = 4           # head groups of 2 per core
VEXT = 97            # v cols: 80 v + 16 pad + ones at 96

F32 = mybir.dt.float32
F32R = mybir.dt.float32r
BF16 = mybir.dt.bfloat16
NPBF16 = mybir.dt.np(BF16)

CANON_SEGS = tuple((SEG * i, SEG * (i + 1)) for i in range(4))

_CACHE: dict = {}


def _segments_from_cu(cu_seqlens: np.ndarray) -> tuple:
    cu = np.asarray(cu_seqlens).astype(np.int64)
    seg = np.searchsorted(cu, np.arange(S), side="right") - 1
    change = np.nonzero(np.diff(seg))[0]
    starts = np.concatenate([[0], change + 1])
    ends = np.concatenate([change + 1, [S]])
    return tuple((int(a), int(b)) for a, b in zip(starts, ends))


def _chan_runs(h: int):
    """(src_row0, dst_ct, dst_row0, n) runs mapping head h's 80 attention
    channels into the 6x128 packed layout at 96-row pitch (16 pad rows per
    head). Runs are 32+32+16 so every src/dst partition base is 32-aligned
    (hardware requires engine AP partition offsets in {0,32,64,96})."""
    def legal(o):
        # engine AP partition rule: size<=32 at any 32-multiple, <=64 at
        # {0,64}, >64 only at 0
        if o == 0:
            return 128
        if o % 64 == 0:
            return 64
        if o % 32 == 0:
            return 32
        return 0
    runs, src = [], 0
    while src < HD:
        gpos = 96 * h + src
        o = gpos % 128
        n = min(legal(o), legal(src % 128), HD - src, 128 - o)
        assert n > 0
        runs.append((src, gpos // 128, o, n))
        src += n
    return runs


def _build(segments, loop_n: int = 1) -> "bacc.Bacc":
    assert segments == CANON_SEGS
    nc = bacc.Bacc("TRN2", target_bir_lowering=False, debug=False,
                   num_devices=NCORES)

    xblk_d = nc.dram_tensor("xblk", [NT, 128, 10, 128], BF16,
                            kind="ExternalInput")
    wqkvb_d = nc.dram_tensor("wqkvb", [GROUPS, 128, 10, 480], BF16,
                             kind="ExternalInput")
    bqkv_d = nc.dram_tensor("bqkv", [1, GROUPS, 480], BF16,
                            kind="ExternalInput")
    cosb_d = nc.dram_tensor("cosb", [128, NT, HD], F32, kind="ExternalInput")
    sinb_d = nc.dram_tensor("sinb", [128, NT, HD], F32, kind="ExternalInput")
    wptb_d = nc.dram_tensor("wptb", [128, 6, DIM], BF16, kind="ExternalInput")
    ident_d = nc.dram_tensor("ident", [128, 128], BF16, kind="ExternalInput")
    ones_d = nc.dram_tensor("onesrow", [1, 128], BF16, kind="ExternalInput")
    onesf_d = nc.dram_tensor("onesf", [1, 128], F32R, kind="ExternalInput")
    out0_d = nc.dram_tensor("outb0", [2, 128, 5, 512], BF16,
                            kind="ExternalOutput")
    out1a_d = nc.dram_tensor("outb1a", [2, 128, 5, 128], BF16,
                             kind="ExternalOutput")
    out1b_d = nc.dram_tensor("outb1b", [2, 128, 5, 128], BF16,
                             kind="ExternalOutput")

    with tile.TileContext(nc) as tc, ExitStack() as ctx:
        if loop_n > 1:
            ctx.enter_context(tc.For_i(0, loop_n, 1))
        per = ctx.enter_context(tc.tile_pool(name="persist", bufs=1))

        ident_sb = per.tile([128, 128], BF16, tag="ident")
        ones_sb = per.tile([1, 128], BF16, tag="ones")
        onesf_sb = per.tile([1, 128], F32R, tag="onesf")
        bqkv_sb = per.tile([1, GROUPS, 480], BF16, tag="bqkv")
        cos_sb = per.tile([128, NT, HD], F32, tag="cos")
        sin_sb = per.tile([128, NT, HD], F32, tag="sin")
        xt_sb = [per.tile([128, 10, 128], BF16, tag=f"xt{t}", name=f"xt{t}")
                 for t in range(NT)]
        wq_sb = [per.tile([128, 10, 480], BF16, tag=f"wq{g}", name=f"wq{g}")
                 for g in range(GROUPS)]
        wp_sb = per.tile([128, 6, DIM], BF16, tag="wp")
        qkTg = [per.tile([HD, 4, SEG], BF16, tag=f"qkT{g}", name=f"qkT{g}")
                for g in range(GROUPS)]
        v_sb = per.tile([128, HPC, NT, VEXT], BF16, tag="v")
        at_sb = per.tile([128, 6, SEG], BF16, tag="at")
        # the 16 pad rows per 96-row head granule are never written by the
        # normalize stage but ARE read by the proj matmul (against zero
        # weights) -- zero the tile so no inf/NaN garbage reaches the PE
        nc.gpsimd.memset(at_sb[:, :, :], 0.0)

        # v pad columns: zeros at 80:96, softmax-denominator ones at 96;
        # memset on the otherwise-idle Pool engine (a broadcast DMA here
        # would emit 6144 tiny descriptors and block the SP queue ~100us)
        nc.gpsimd.memset(v_sb[:, :, :, HD:VEXT - 1], 0.0)
        nc.gpsimd.memset(v_sb[:, :, :, VEXT - 1:VEXT], 1.0)

        # input DMAs: the HWDGE issue slot is a single shared ~625ns/DMA
        # resource, so use few dense DMAs, ordered by first use
        nc.sync.dma_start(out=xt_sb[0], in_=xblk_d[0])
        nc.sync.dma_start(out=wq_sb[0][:, 0:5, :], in_=wqkvb_d[0, :, 0:5, :])
        nc.sync.dma_start(out=wq_sb[0][:, 5:10, :], in_=wqkvb_d[0, :, 5:10, :])
        nc.sync.dma_start(out=xt_sb[1], in_=xblk_d[1])
        nc.sync.dma_start(out=ones_sb, in_=ones_d[:, :])
        nc.sync.dma_start(out=bqkv_sb, in_=bqkv_d[:, :, :])
        nc.sync.dma_start(out=xt_sb[2], in_=xblk_d[2])
        nc.sync.dma_start(out=cos_sb, in_=cosb_d[:, :, :])
        nc.sync.dma_start(out=sin_sb, in_=sinb_d[:, :, :])
        nc.sync.dma_start(out=xt_sb[3], in_=xblk_d[3])
        nc.sync.dma_start(out=ident_sb, in_=ident_d[:, :])
        nc.sync.dma_start(out=xt_sb[4], in_=xblk_d[4])
        nc.sync.dma_start(out=xt_sb[5], in_=xblk_d[5])
        nc.sync.dma_start(out=wq_sb[1], in_=wqkvb_d[1])
        nc.sync.dma_start(out=onesf_sb, in_=onesf_d[:, :])
        nc.sync.dma_start(out=wq_sb[2], in_=wqkvb_d[2])
        nc.sync.dma_start(out=wq_sb[3], in_=wqkvb_d[3])
        nc.sync.dma_start(out=wp_sb, in_=wptb_d[:, :, :])

        qpp = ctx.enter_context(tc.tile_pool(name="qpp", bufs=2, space="PSUM"))
        tpp = ctx.enter_context(tc.tile_pool(name="tpp", bufs=1, space="PSUM"))
        scp = ctx.enter_context(tc.tile_pool(name="scp", bufs=3, space="PSUM"))
        app = ctx.enter_context(tc.tile_pool(name="app", bufs=2, space="PSUM"))
        ropep = ctx.enter_context(tc.tile_pool(name="ropet", bufs=4))
        qkrop = ctx.enter_context(tc.tile_pool(name="qkro", bufs=4))
        expp = ctx.enter_context(tc.tile_pool(name="expp", bufs=8))
        smp = ctx.enter_context(tc.tile_pool(name="smalls", bufs=4))
        outp = ctx.enter_context(tc.tile_pool(name="outp", bufs=3))

        def emit_tp(g, t, ro):
            tp = tpp.tile([HD, 4, 128], BF16, tag="tp", name="tpps")
            for j in range(4):
                nc.tensor.transpose(tp[:, j, :], ro[:, HD * j:HD * (j + 1)],
                                    ident_sb)
            nc.vector.tensor_copy(qkTg[g][:, :, 128 * t:128 * (t + 1)], tp)

        def emit_A(g):
            """qkv + rope + v copy + qk transposes for head group g;
            yields once per s-tile so B work of the previous group can be
            interleaved between tiles.

            The transposes for tile t are deferred until after tile t+1's
            qkv matmuls so the PE never waits on the DVE RoPE chain."""
            pending = None
            for t in range(NT):
                qp = qpp.tile([128, 480], F32, tag="qp", name="qkvps")
                for dp in range(5):
                    nc.tensor.matmul(qp[:, :], lhsT=xt_sb[t][:, dp, :],
                                     rhs=wq_sb[g][:, dp, :],
                                     start=(dp == 0), stop=False)
                yield
                for dp in range(5, 10):
                    nc.tensor.matmul(qp[:, :], lhsT=xt_sb[t][:, dp, :],
                                     rhs=wq_sb[g][:, dp, :],
                                     start=False, stop=False)
                nc.tensor.matmul(qp[:, :], lhsT=ones_sb[:, :],
                                 rhs=bqkv_sb[:, g, :], start=False, stop=True)
                if pending is not None:
                    emit_tp(g, *pending)

                m1 = ropep.tile([128, 320], BF16, tag="m1")
                m2 = ropep.tile([128, 320], BF16, tag="m2")
                qk_h = qp[:, 0:320].rearrange("p (h d) -> p h d", h=4)
                cos_b = cos_sb[:, t:t + 1, :].to_broadcast([128, 4, HD])
                with nc.allow_low_precision("bf16 matmul inputs"):
                    nc.vector.tensor_mul(
                        m1.rearrange("p (h d) -> p h d", h=4), qk_h, cos_b)
                swap = qp[:, 0:320].rearrange(
                    "p (h x d) -> p h x d", h=4, x=2)[:, :, ::-1, :]
                sin_b = sin_sb[:, t:t + 1, :].rearrange(
                    "p t (x d) -> p (t x) d", x=2)[:, None, :, :] \
                    .to_broadcast([128, 4, 2, HD // 2])
                with nc.allow_low_precision("bf16 matmul inputs"):
                    nc.vector.tensor_mul(
                        m2.rearrange("p (h x d) -> p h x d", h=4, x=2),
                        swap, sin_b)
                ro = qkrop.tile([128, 320], BF16, tag="qkro")
                with nc.allow_low_precision("bf16 matmul inputs"):
                    nc.vector.tensor_add(ro, m1, m2)

                with nc.allow_low_precision("bf16 matmul inputs"):
                    nc.scalar.copy(
                        v_sb[:, 2 * g:2 * g + 2, t, 0:HD],
                        qp[:, 320:480].rearrange("p (e d) -> p e d", e=2))
                pending = (t, ro)
                yield
            emit_tp(g, *pending)

        def emit_B(h, qc0, qc1):
            """attention for core-local head h over q columns [qc0, qc1);
            yields between pipeline stages."""
            g, e = h // 2, h % 2
            qT = qkTg[g][:, e]
            kT = qkTg[g][:, 2 + e]
            qna = qc1 - qc0
            ap_ = app.tile([VEXT, 512], F32, tag="ap", name="attps")
            blocks = list(range(NT))
            for g0 in range(0, NT, 3):
                grp = blocks[g0:g0 + 3]
                exs = []
                for j in grp:
                    sc = scp.tile([128, 512], F32, tag="sc", name="scps")
                    nc.tensor.matmul(sc[:, :qna],
                                     lhsT=kT[:, 128 * j:128 * (j + 1)],
                                     rhs=qT[:, qc0:qc1],
                                     start=True, stop=True)
                    ex = expp.tile([128, 512], BF16, tag="expp")
                    nc.scalar.activation(ex[:, :qna], sc[:, :qna],
                                         mybir.ActivationFunctionType.Exp)
                    exs.append(ex)
                yield
                for j, ex in zip(grp, exs):
                    nc.tensor.matmul(ap_[:, :qna], lhsT=v_sb[:, h, j, :],
                                     rhs=ex[:, :qna],
                                     start=(j == 0), stop=(j == NT - 1))
                yield
            den = smp.tile([1, 512], F32R, tag="den", name="den")
            with nc.allow_low_precision("f32r matmul inputs"):
                if qna == 512:
                    nc.scalar.copy(den[:, :qna], ap_[96:97, :qna])
                else:
                    nc.vector.tensor_copy(den[:, :qna], ap_[96:97, :qna])
            yield
            bc = scp.tile([HD, 512], F32, tag="sc", name="bcps")
            nc.tensor.matmul(bc[:, :qna], lhsT=onesf_sb[:, 0:HD],
                             rhs=den[:, :qna], start=True, stop=True)
            rec = smp.tile([HD, 512], F32, tag="rec", name="rec")
            nc.vector.reciprocal(rec[:, :qna], bc[:, :qna])
            for (src, ct, dst, n) in _chan_runs(h):
                with nc.allow_low_precision("bf16 matmul inputs"):
                    nc.vector.tensor_mul(
                        at_sb[dst:dst + n, ct, qc0:qc1],
                        ap_[src:src + n, :qna],
                        rec[src:src + n, :qna])
            yield

        def emit_C(c0, c1, out_d, finer=False):
            """proj for s columns [c0, c1); dense output DMA per half-dim
            chunk (or per m-tile when finer), issue alternating SP/ACT."""
            n = c1 - c0
            for mh in range(2):
                ob = outp.tile([128, 5, n], BF16, tag="outp")
                for mm in range(5):
                    m = 5 * mh + mm
                    pp = qpp.tile([128, 512], F32, tag="qp", name="prps")
                    for ct in range(6):
                        nc.tensor.matmul(
                            pp[:, :n],
                            lhsT=wp_sb[:, ct, 128 * m:128 * (m + 1)],
                            rhs=at_sb[:, ct, c0:c1],
                            start=(ct == 0), stop=(ct == 5))
                    with nc.allow_low_precision("bf16 output"):
                        nc.scalar.copy(ob[:, mm, :], pp[:, :n])
                    if finer:
                        nc.sync.dma_start(
                            out=out_d[mh, :, mm, :], in_=ob[:, mm, :])
                    yield
                if not finer:
                    nc.sync.dma_start(out=out_d[mh], in_=ob)

        def chain(*gens):
            for gg in gens:
                yield from gg

        def zipgen(gen_a, gen_b):
            """Alternate single steps of two independent streams; drains
            both. Used for head pairs so one head's PE stages fill the
            other's exp-latency holes."""
            a_live, b_live = True, True
            while a_live or b_live:
                if a_live:
                    a_live = next(gen_a, StopIteration) is not StopIteration
                if b_live:
                    b_live = next(gen_b, StopIteration) is not StopIteration
                yield

        def interleave(gen_a, gen_b, ratio):
            """Alternate: one step of gen_a, then `ratio` steps of gen_b.
            Drains both."""
            a_live, b_live = True, True
            while a_live or b_live:
                if a_live:
                    a_live = next(gen_a, StopIteration) is not StopIteration
                if b_live:
                    for _ in range(ratio):
                        if next(gen_b, StopIteration) is StopIteration:
                            b_live = False
                            break

        def gen_group_B(g):
            for e in range(2):
                h = 2 * g + e
                yield from emit_B(h, 0, 512)
                yield from emit_B(h, 512, SEG)

        prev_b = None
        for g in range(GROUPS):
            if prev_b is None:
                for _ in emit_A(g):
                    pass
            else:
                interleave(emit_A(g), prev_b, 4)
            if g < GROUPS - 1:
                prev_b = gen_group_B(g)
        # tail: last group's B with C interleaved once its inputs are ready;
        # the last head's second chunk is split so the final serial
        # norm->proj->DMA chain covers only 128 columns
        h0, h1 = 2 * (GROUPS - 1), 2 * (GROUPS - 1) + 1
        for _ in zipgen(emit_B(h0, 0, 512), emit_B(h1, 0, 512)):
            pass
        interleave(chain(emit_B(h0, 512, SEG), emit_B(h1, 512, 640),
                         emit_B(h1, 640, SEG)),
                   emit_C(0, 512, out0_d), 3)
        for _ in emit_C(512, 640, out1a_d):
            pass
        for _ in emit_C(640, SEG, out1b_d):
            pass

    nc.compile()
    return nc


def _prep_inputs(x, cu_seqlens, rotary_pos_emb, w_qkv, b_qkv, w_proj, b_proj):
    """Host-side shard prep. Returns per-core input dicts."""
    scale = np.float32(1.0 / np.sqrt(np.float32(HD)))
    x = np.asarray(x, np.float32)
    w_qkv = np.asarray(w_qkv, np.float32)
    b_qkv = np.asarray(b_qkv, np.float32)
    w_proj = np.asarray(w_proj, np.float32)
    rot = np.asarray(rotary_pos_emb, np.float32)

    cosw = np.concatenate([np.cos(rot), np.cos(rot)], axis=1)
    sinw = np.concatenate([-np.sin(rot), np.sin(rot)], axis=1)

    ident = np.eye(128, dtype=NPBF16)
    onesrow = np.ones((1, 128), dtype=NPBF16)
    onesf = np.ones((1, 128), dtype=np.float32)

    in_maps = []
    for c in range(NCORES):
        si, hh = c // 2, c % 2
        s0 = SEG * si
        heads = list(range(8 * hh, 8 * hh + 8))

        xs = x[s0:s0 + SEG].astype(NPBF16)  # [768, 1280]
        # xblk[t, p, dp, s'] = x[s0+128t+s', 128dp+p]
        xblk = np.ascontiguousarray(
            xs.reshape(NT, 128, 10, 128).transpose(0, 3, 2, 1))

        # w_qkv rows in per-group order [q_a q_b k_a k_b v_a v_b] x 80
        idx = []
        for g in range(GROUPS):
            a, b = heads[2 * g], heads[2 * g + 1]
            for base, hsel in ((0, a), (0, b), (DIM, a), (DIM, b),
                               (2 * DIM, a), (2 * DIM, b)):
                idx.extend(range(base + hsel * HD, base + (hsel + 1) * HD))
        w_c = w_qkv[idx, :].copy()
        b_c = b_qkv[idx].copy()
        for g in range(GROUPS):
            w_c[480 * g:480 * g + 160] *= scale
            b_c[480 * g:480 * g + 160] *= scale
        # wqkvb[g, p, dp, cc] = w_c[480g+cc, 128dp+p] (dense per-group)
        wqkvb = np.ascontiguousarray(
            w_c.T.reshape(10, 128, GROUPS, 480).transpose(2, 1, 0, 3)
        ).astype(NPBF16)
        bqkvb = np.ascontiguousarray(b_c.reshape(1, GROUPS, 480)).astype(NPBF16)

        # rope tables [128, 6, 80] for this segment's rows
        cosb = np.ascontiguousarray(
            cosw[s0:s0 + SEG].reshape(NT, 128, HD).transpose(1, 0, 2))
        sinb = np.ascontiguousarray(
            sinw[s0:s0 + SEG].reshape(NT, 128, HD).transpose(1, 0, 2))

        # wptb[ct, p, m] = w_proj[m, chan(128ct+p)], chan c -> head
        # heads[c//80], dim c%80
        wptb = np.zeros((6, 128, DIM), np.float32)
        wv = wptb.reshape(768, DIM)
        for hl, habs in enumerate(heads):
            wv[96 * hl:96 * hl + HD] = w_proj[:, habs * HD:(habs + 1) * HD].T
        wptb = np.ascontiguousarray(
            wptb.transpose(1, 0, 2)).astype(NPBF16)  # [128, 6, DIM]

        in_maps.append({
            "xblk": xblk,
            "wqkvb": wqkvb,
            "bqkv": bqkvb,
            "cosb": cosb,
            "sinb": sinb,
            "wptb": np.ascontiguousarray(wptb),
            "ident": ident,
            "onesrow": onesrow,
            "onesf": onesf,
        })
    return in_maps


def run(inputs: dict, trace: bool = False):
    segments = _segments_from_cu(inputs["cu_seqlens"])
    if segments != CANON_SEGS:
        return _legacy_run(inputs, trace=trace)
    key = (segments, "v2")
    if key not in _CACHE:
        _CACHE[key] = _build(segments)
    nc = _CACHE[key]
    in_maps = _prep_inputs(
        inputs["x"], inputs["cu_seqlens"], inputs["rotary_pos_emb"],
        inputs["w_qkv"], inputs["b_qkv"], inputs["w_proj"], inputs["b_proj"])
    res = run_bass_kernel_spmd(nc, in_maps, core_ids=list(range(NCORES)),
                               trace=trace)
    acc = np.zeros((DIM, S), np.float64)
    for c, r in enumerate(res.results):
        si = c // 2
        part = np.zeros((128, 10, SEG), np.float64)
        for mh in range(2):
            part[:, 5 * mh:5 * (mh + 1), 0:512] = r["outb0"][mh]
            part[:, 5 * mh:5 * (mh + 1), 512:640] = r["outb1a"][mh]
            part[:, 5 * mh:5 * (mh + 1), 640:768] = r["outb1b"][mh]
        # partial[128m+p, s'] = part[p, m, s']
        acc[:, SEG * si:SEG * (si + 1)] += part.transpose(1, 0, 2).reshape(
            DIM, SEG)
    acc += np.asarray(inputs["b_proj"], np.float64)[:, None]
    out = np.ascontiguousarray(acc.T.astype(np.float32))
    return out, res


def kernel(**inputs) -> np.ndarray:
    out, _ = run(inputs, trace=False)
    return out


_LEGACY_MOD = None


def _legacy_run(inputs: dict, trace: bool = False):
    """General-segment fallback: the original heads-only-sharded kernel,
    embedded so kernel.py stays self-contained. Only used when cu_seqlens
    does not produce four 768-row segments."""
    global _LEGACY_MOD
    if _LEGACY_MOD is None:
        import types
        _LEGACY_MOD = types.ModuleType("kernel_legacy_embedded")
        exec(_LEGACY_SRC, _LEGACY_MOD.__dict__)
    return _LEGACY_MOD.run(inputs, trace=trace)


_LEGACY_SRC = r'''
"""Trainium2 Bass kernel for nn_Attention_72541997629647 (sparse varlen attention).

Computation (see problem reference):
  qkv = x @ w_qkv.T + b_qkv ; NeoX RoPE on q,k ; block-diagonal softmax
  attention from cu_seqlens segments ; out = (attn @ v) @ w_proj.T + b_proj

Sharding: tensor-parallel over heads. 16 heads / 8 cores = 2 heads per core.
Each core computes q/k/v for its 2 heads, runs block-diagonal attention, and
produces a partial projection output (full [DIM, S], transposed); the host
sums the 8 partials and adds b_proj, so the result is exact.

Device dataflow per core (all matmuls in float32r: full fp32 storage,
reduced-precision multiply at 4x the fp32 matmul rate):
  A) QKV: out_nat[s, 480] = xT-chunks.T @ w_chunks (+ bias via ones-row
     matmul); RoPE applied on the free dim (half-swap via negative-step AP,
     sign folded into the host-built sin table); q,k PE-transposed to
     [hd, S]; v kept natural with an appended ones column (denominator trick).
  B) per (head, segment, q-chunk): scoresT[k,q] = kT-block.T @ qT ; exp on
     ACT ; attn_extT[81, q] += v_ext.T @ exp accumulated over k-blocks; row 80
     is the softmax denominator. normalize = reciprocal + ones-matmul
     partition-broadcast + multiply.
  C) proj: outT[dim, s] += wpT-head.T @ attn_outT-head ; PSUM->SBUF copy on
     DVE; output written as fully-contiguous 1.25MB blocks (one dense
     descriptor chain per DMA, ~70us faster than 2KB-strided rows) and
     unscrambled on the host. b_proj is added host-side.
"""

import os
import sys

for _p in ("/opt/trn_rl_repo", "/root/.axon_site/_ro/trn_rl_repo"):
    if os.path.isdir(_p) and _p not in sys.path:
        sys.path.insert(0, _p)

import numpy as np

import concourse.bacc as bacc
import concourse.bass as bass
import concourse.mybir as mybir
import concourse.tile as tile
from concourse.bass_utils import run_bass_kernel_spmd
from contextlib import ExitStack

S = 3072
DIM = 1280
H = 16
HD = 80
NCORES = 8
HPC = H // NCORES          # heads per core = 2
QKDIM = 2 * HPC * HD       # 320 (q+k outdims per core)
ODIM = 3 * HPC * HD        # 480 (qkv outdims per core)
CDIM = HPC * HD            # 160 (attn channels per core)

F32 = mybir.dt.float32
F32R = mybir.dt.float32r
MM_DT = F32R               # matmul input dtype (F32R: 4x faster, ~1e-4 rel err)

_CACHE: dict = {}


def _segments_from_cu(cu_seqlens: np.ndarray) -> tuple:
    """Contiguous runs of equal segment id, exactly as the reference's
    searchsorted-based mask defines them."""
    cu = np.asarray(cu_seqlens).astype(np.int64)
    seg = np.searchsorted(cu, np.arange(S), side="right") - 1
    change = np.nonzero(np.diff(seg))[0]
    starts = np.concatenate([[0], change + 1])
    ends = np.concatenate([change + 1, [S]])
    return tuple((int(a), int(b)) for a, b in zip(starts, ends))


def _build(segments, loop_n: int = 1) -> "bacc.Bacc":
    nc = bacc.Bacc("TRN2", target_bir_lowering=False, debug=False,
                   num_devices=NCORES)

    xblk_d = nc.dram_tensor("xblk", [S // 512, 5, 128, 2, 512], MM_DT,
                        kind="ExternalInput")
    wqkvT_d = nc.dram_tensor("wqkvT", [DIM, ODIM], MM_DT, kind="ExternalInput")
    bqkv_d = nc.dram_tensor("bqkv", [1, ODIM], MM_DT, kind="ExternalInput")
    cosb_d = nc.dram_tensor("cosb", [S // 512, 128, 4, HD], F32,
                        kind="ExternalInput")
    sinb_d = nc.dram_tensor("sinb", [S // 512, 128, 4, HD], F32,
                        kind="ExternalInput")
    wpT_d = nc.dram_tensor("wpT", [CDIM, DIM], MM_DT, kind="ExternalInput")
    ident_d = nc.dram_tensor("ident", [128, 128], MM_DT, kind="ExternalInput")
    ones_d = nc.dram_tensor("onesrow", [1, 128], MM_DT, kind="ExternalInput")
    vpad_d = nc.dram_tensor("vpad", [17], MM_DT, kind="ExternalInput")
    # boundary-block 0/1 masks (segments not aligned to the 128 grid);
    # order must match the (head-agnostic) traversal below.
    bpairs = []
    for (s0, s1) in segments:
        for j in range(s0 // 128, -(-s1 // 128)):
            r0, r1 = max(0, s0 - 128 * j), min(128, s1 - 128 * j)
            if r0 > 0 or r1 < 128:
                bpairs.append((j, r0, r1))
    nbm = len(bpairs)
    bmask_d = (nc.dram_tensor("bmask", [nbm, 128], MM_DT, kind="ExternalInput")
               if nbm else None)
    outb_d = nc.dram_tensor("outb", [S // 512, 2, 128, 5, 512], F32,
                        kind="ExternalOutput")

    NT = S // 128   # 24 s-tiles
    NSS = S // 512  # 6 s-superchunks

    with tile.TileContext(nc) as tc, ExitStack() as ctx:
        if loop_n > 1:  # benchmarking only: repeat the whole body on-device
            ctx.enter_context(tc.For_i(0, loop_n, 1))
        per = ctx.enter_context(tc.tile_pool(name="persist", bufs=1))

        # small constants first so nothing cheap blocks the pipeline
        bqkv_sb = per.tile([1, ODIM], MM_DT, tag="bqkv")
        nc.sync.dma_start(out=bqkv_sb, in_=bqkv_d[:, :])
        ident_sb = per.tile([128, 128], MM_DT, tag="ident")
        nc.sync.dma_start(out=ident_sb, in_=ident_d[:, :])
        ones_sb = per.tile([1, 128], MM_DT, tag="ones")
        nc.sync.dma_start(out=ones_sb, in_=ones_d[:, :])
        # per-d-chunk qkv weights and per-superchunk rope tables: split so the
        # first matmul/rope can start after a fraction of the weight traffic
        wqkv_sb = [per.tile([128, ODIM], MM_DT, tag=f"wqkv{d}", name=f"wqkv{d}")
                   for d in range(10)]
        cos_sb = [per.tile([128, 4, HD], F32, tag=f"cos{ss}", name=f"cos{ss}")
                  for ss in range(NSS)]
        sin_sb = [per.tile([128, 4, HD], F32, tag=f"sin{ss}", name=f"sin{ss}")
                  for ss in range(NSS)]
        wp_sb = [per.tile([HD, DIM], MM_DT, tag=f"wp{h}", name=f"wp{h}") for h in range(HPC)]
        for h in range(HPC):
            nc.sync.dma_start(out=wp_sb[h], in_=wpT_d[h * HD:(h + 1) * HD, :])

        # v extended to 97 cols: 80 v-dims, 16 zero pad, ones col at 96 so the
        # denominator lands on a 32-aligned PSUM partition. Split per 512-s
        # superchunk so attention can start before all of phase A finishes.
        VEXT = 97
        v_sb = [[per.tile([128, 4, VEXT], MM_DT, tag=f"v{h}_{ss}",
                          name=f"v{h}_{ss}") for ss in range(NSS)]
                for h in range(HPC)]
        qkT = [[per.tile([HD, 512], MM_DT, tag=f"qkT{j}_{ss}",
                         name=f"qkT{j}_{ss}") for ss in range(NSS)]
               for j in range(2 * HPC)]
        att_o = [[per.tile([HD, 512], MM_DT, tag=f"atto{h}_{ss}",
                           name=f"atto{h}_{ss}") for ss in range(NSS)]
                 for h in range(HPC)]

        # one shared PSUM pool (8 bank-sized slots shared by every phase so
        # the scheduler can overlap A/B/C), plus top-level SBUF pools
        psp = ctx.enter_context(tc.tile_pool(name="ps", bufs=8, space="PSUM"))
        xtp = ctx.enter_context(tc.tile_pool(name="xt", bufs=6))
        ropep = ctx.enter_context(tc.tile_pool(name="ropet", bufs=2))
        qkrop = ctx.enter_context(tc.tile_pool(name="qkro", bufs=3))
        expp = ctx.enter_context(tc.tile_pool(name="expp", bufs=5))
        smp = ctx.enter_context(tc.tile_pool(name="smalls", bufs=2))
        outp = ctx.enter_context(tc.tile_pool(name="outp", bufs=2))

        if nbm:
            bmask_sb = per.tile([128, nbm], MM_DT, tag="bmask")
            nc.sync.dma_start(out=bmask_sb,
                              in_=bmask_d.ap().rearrange("n p -> p n"))
            bidx = {(j, r0, r1): i for i, (j, r0, r1) in enumerate(bpairs)}

        # ---------------- phase bodies (emitted interleaved below) --------
        def emit_A(ss):
            """QKV + RoPE + transposes for s-superchunk ss."""
            xts = []
            for dp in range(5):
                if ss == 0:
                    for d in (2 * dp, 2 * dp + 1):
                        nc.sync.dma_start(
                            out=wqkv_sb[d],
                            in_=wqkvT_d[128 * d:128 * (d + 1), :])
                xt = xtp.tile([128, 2, 512], MM_DT, tag="xt", name="xt")
                nc.sync.dma_start(out=xt, in_=xblk_d[ss, dp])
                xts.append(xt)
            nc.sync.dma_start(out=cos_sb[ss], in_=cosb_d[ss])
            nc.sync.dma_start(out=sin_sb[ss], in_=sinb_d[ss])
            for h in range(HPC):
                nc.sync.dma_start(
                    out=v_sb[h][ss][:, :, HD:VEXT],
                    in_=bass.AP(tensor=vpad_d, offset=0,
                                ap=[[0, 128], [0, 4], [1, VEXT - HD]]))
            tp_ps = [psp.tile([HD, 512], MM_DT, tag="ps", name="tpps")
                     for _ in range(2 * HPC)]
            nh = 2 * HPC  # 4 roped qk tensor-heads
            for sub in range(4):
                qp = psp.tile([128, ODIM], F32, tag="ps", name="qkvps")
                for d in range(10):
                    nc.tensor.matmul(
                        qp[:, :],
                        lhsT=xts[d // 2][:, d % 2, 128 * sub:128 * (sub + 1)],
                        rhs=wqkv_sb[d], start=(d == 0), stop=False)
                nc.tensor.matmul(qp[:, :], lhsT=ones_sb[:, :],
                                 rhs=bqkv_sb[:, :], start=False, stop=True)

                # RoPE over q,k: out = t*cos + halfswap(t)*sinsgn
                m1 = ropep.tile([128, QKDIM], F32, tag="m1")
                m2 = ropep.tile([128, QKDIM], F32, tag="m2")
                qk_h = qp[:, 0:QKDIM].rearrange("p (h d) -> p h d", h=nh)
                cos_b = cos_sb[ss][:, sub:sub + 1, :].to_broadcast(
                    [128, nh, HD])
                nc.vector.tensor_mul(
                    m1.rearrange("p (h d) -> p h d", h=nh), qk_h, cos_b)
                swap = qp[:, 0:QKDIM].rearrange(
                    "p (h x d) -> p h x d", h=nh, x=2)[:, :, ::-1, :]
                sin_b = sin_sb[ss][:, sub:sub + 1, :].rearrange(
                    "p t (x d) -> p (t x) d", x=2)[:, None, :, :] \
                    .to_broadcast([128, nh, 2, HD // 2])
                nc.vector.tensor_mul(
                    m2.rearrange("p (h x d) -> p h x d", h=nh, x=2),
                    swap, sin_b)
                ro = qkrop.tile([128, QKDIM], MM_DT, tag="qkro")
                with nc.allow_low_precision("f32r matmul inputs"):
                    nc.vector.tensor_add(ro, m1, m2)

                # v natural copy (its bias already in psum)
                for h in range(HPC):
                    nc.scalar.copy(
                        v_sb[h][ss][:, sub, 0:HD],
                        qp[:, QKDIM + HD * h:QKDIM + HD * (h + 1)])

                # transpose roped q,k to [hd, s]
                for j in range(2 * HPC):
                    nc.tensor.transpose(
                        tp_ps[j][:, 128 * sub:128 * (sub + 1)],
                        ro[:, HD * j:HD * (j + 1)], ident_sb)
            for j in range(2 * HPC):
                nc.scalar.copy(qkT[j][ss], tp_ps[j])

        def emit_B(seg):
            """block-diagonal attention for one segment (both heads)."""
            s0, s1 = seg
            jb0, jb1 = s0 // 128, -(-s1 // 128)
            # q-chunks aligned to the 512 grid so each lives in one tile
            g = (s0 // 512) * 512
            qchunks = []
            while g < s1:
                q0, q1 = max(s0, g), min(s1, g + 512)
                if q1 > q0:
                    qchunks.append((q0, q1))
                g += 512
            for q0, q1 in qchunks:
                qn = q1 - q0
                ss_q, c0 = q0 // 512, q0 % 512
                # fp32r matmuls need an even/4-aligned moving dim: widen the
                # compute window to 4-aligned columns (scratch cols unread)
                qa0 = q0 - (q0 % 4)
                qa1 = min(512 * (ss_q + 1), q1 + ((-q1) % 4))
                qna, off, ca0 = qa1 - qa0, q0 - qa0, qa0 % 512
                for h in range(HPC):
                    ap_ = psp.tile([VEXT, 512], F32, tag="ps", name="attps")
                    blocks = list(range(jb0, jb1))
                    for g0 in range(0, len(blocks), 4):
                        grp = blocks[g0:g0 + 4]
                        exs = []
                        for j in grp:
                            kTt = qkT[HPC + h][j // 4]
                            sc = psp.tile([128, 512], F32, tag="ps",
                                          name="scps")
                            nc.tensor.matmul(
                                sc[:, :qna],
                                lhsT=kTt[:, 128 * (j % 4):128 * (j % 4 + 1)],
                                rhs=qkT[h][ss_q][:, ca0:ca0 + qna],
                                start=True, stop=True)
                            ex = expp.tile([128, 512], MM_DT, tag="expp")
                            nc.scalar.activation(
                                ex[:, :qna], sc[:, :qna],
                                mybir.ActivationFunctionType.Exp)
                            r0, r1 = max(0, s0 - 128 * j), min(128, s1 - 128 * j)
                            if r0 > 0 or r1 < 128:
                                # zero out-of-segment rows of this block
                                mi = bidx[(j, r0, r1)]
                                with nc.allow_low_precision("f32r inputs"):
                                    nc.vector.tensor_mul(
                                        ex[:, :qna], ex[:, :qna],
                                        bmask_sb[:, mi:mi + 1]
                                        .to_broadcast([128, qna]))
                            exs.append(ex)
                        for j, ex in zip(grp, exs):
                            nc.tensor.matmul(
                                ap_[:, :qna],
                                lhsT=v_sb[h][j // 4][:, j % 4, :],
                                rhs=ex[:, :qna],
                                start=(j == blocks[0]),
                                stop=(j == blocks[-1]))
                    den = smp.tile([1, 512], MM_DT, tag="den", name="den")
                    with nc.allow_low_precision("f32r matmul inputs"):
                        nc.scalar.copy(den[:, :qna], ap_[96:97, :qna])
                    bc = psp.tile([HD, 512], F32, tag="ps", name="bcps")
                    nc.tensor.matmul(bc[:, :qna], lhsT=ones_sb[:, 0:HD],
                                     rhs=den[:, :qna], start=True, stop=True)
                    rec = smp.tile([HD, 512], F32, tag="rec", name="rec")
                    nc.vector.reciprocal(rec[:, :qna], bc[:, :qna])
                    with nc.allow_low_precision("f32r matmul inputs"):
                        nc.vector.tensor_mul(att_o[h][ss_q][:, c0:c0 + qn],
                                             ap_[0:HD, off:off + qn],
                                             rec[:, off:off + qn])

        def emit_C(sc_):
            """projection for output s-superchunk sc_.
            b_proj is added host-side after the cross-core partial sum."""
            for mh in range(2):
                ob = outp.tile([128, 5, 512], F32, tag="outp")
                for mm_ in range(5):
                    m = 5 * mh + mm_
                    pp = psp.tile([128, 512], F32, tag="ps", name="prps")
                    for h in range(HPC):
                        nc.tensor.matmul(
                            pp[:, :],
                            lhsT=wp_sb[h][:, 128 * m:128 * (m + 1)],
                            rhs=att_o[h][sc_],
                            start=(h == 0), stop=(h == HPC - 1))
                    nc.vector.tensor_copy(ob[:, mm_, :], pp)
                nc.sync.dma_start(out=outb_d[sc_, mh], in_=ob)

        # ---- interleaved driver: emit B as soon as its span is produced,
        # ---- C as soon as all segments covering its chunk are attended.
        segs_left = sorted(segments, key=lambda s: s[1])
        segs_done: list = []
        c_next = 0
        for ss in range(NSS):
            emit_A(ss)
            done_to = 512 * (ss + 1)
            while segs_left and segs_left[0][1] <= done_to:
                seg = segs_left.pop(0)
                emit_B(seg)
                segs_done.append(seg)
            covered = min((s0 for (s0, s1) in segs_left), default=S)
            while c_next < NSS and 512 * (c_next + 1) <= covered:
                emit_C(c_next)
                c_next += 1
        assert not segs_left
        while c_next < NSS:
            emit_C(c_next)
            c_next += 1

    nc.compile()
    return nc


def _prep_inputs(x, cu_seqlens, rotary_pos_emb, w_qkv, b_qkv, w_proj, b_proj):
    """Host-side shard prep. Returns per-core input dicts."""
    scale = np.float32(1.0 / np.sqrt(np.float32(HD)))
    xT = np.ascontiguousarray(np.asarray(x, np.float32).T)
    w_qkv = np.asarray(w_qkv, np.float32)
    b_qkv = np.asarray(b_qkv, np.float32)
    w_proj = np.asarray(w_proj, np.float32)
    b_proj = np.asarray(b_proj, np.float32)
    rot = np.asarray(rotary_pos_emb, np.float32)

    cosw = np.concatenate([np.cos(rot), np.cos(rot)], axis=1).astype(np.float32)
    sinw = np.concatenate([-np.sin(rot), np.sin(rot)], axis=1).astype(np.float32)
    # blocked layouts so every device DMA reads one dense contiguous region:
    # xblk[ss,dp,p,c,n] = xT[256dp+128c+p, 512ss+n]; cosb[ss,p,t,d] likewise
    xblk = np.ascontiguousarray(
        xT.reshape(5, 2, 128, 6, 512).transpose(3, 0, 2, 1, 4))
    cosb = np.ascontiguousarray(
        cosw.reshape(6, 4, 128, HD).transpose(0, 2, 1, 3))
    sinb = np.ascontiguousarray(
        sinw.reshape(6, 4, 128, HD).transpose(0, 2, 1, 3))
    ident = np.eye(128, dtype=np.float32)
    onesrow = np.ones((1, 128), dtype=np.float32)
    vpad = np.zeros(17, dtype=np.float32)
    vpad[16] = 1.0
    segments = _segments_from_cu(cu_seqlens)
    bmask_rows = []
    for (s0, s1) in segments:
        for j in range(s0 // 128, -(-s1 // 128)):
            r0, r1 = max(0, s0 - 128 * j), min(128, s1 - 128 * j)
            if r0 > 0 or r1 < 128:
                row = np.zeros(128, dtype=np.float32)
                row[r0:r1] = 1.0
                bmask_rows.append(row)
    bmask = np.stack(bmask_rows) if bmask_rows else None

    in_maps = []
    for c in range(NCORES):
        heads = [HPC * c + i for i in range(HPC)]
        idx = []
        for base in (0, DIM, 2 * DIM):           # q, k, v row blocks
            for h in heads:
                idx.extend(range(base + h * HD, base + (h + 1) * HD))
        w_c = w_qkv[idx, :].copy()
        b_c = b_qkv[idx].copy()
        w_c[:QKDIM // 2] *= scale                # scale q by 1/sqrt(HD)
        b_c[:QKDIM // 2] *= scale
        cdims = []
        for h in heads:
            cdims.extend(range(h * HD, (h + 1) * HD))
        wpT = np.ascontiguousarray(w_proj[:, cdims].T)  # [CDIM, DIM]
        in_maps.append({
            "xblk": xblk,
            "wqkvT": np.ascontiguousarray(w_c.T),
            "bqkv": np.ascontiguousarray(b_c[None, :]),
            "cosb": cosb,
            "sinb": sinb,
            "wpT": wpT,
            "ident": ident,
            "onesrow": onesrow,
            "vpad": vpad,
        })
        if bmask is not None:
            in_maps[-1]["bmask"] = bmask
    return in_maps


def run(inputs: dict, trace: bool = False):
    """Build (cached), run on 8 cores, return (out [S, DIM] fp32, results)."""
    segments = _segments_from_cu(inputs["cu_seqlens"])
    key = (segments, str(MM_DT))
    if key not in _CACHE:
        _CACHE[key] = _build(segments)
    nc = _CACHE[key]
    in_maps = _prep_inputs(
        inputs["x"], inputs["cu_seqlens"], inputs["rotary_pos_emb"],
        inputs["w_qkv"], inputs["b_qkv"], inputs["w_proj"], inputs["b_proj"])
    res = run_bass_kernel_spmd(nc, in_maps, core_ids=list(range(NCORES)),
                               trace=trace)
    acc = np.zeros((DIM, S), np.float64)
    for r in res.results:
        # blocked [sc, mh, p, c, n] -> [dim = 640*mh+128*c+p, s = 512*sc+n]
        acc += r["outb"].transpose(1, 3, 2, 0, 4).reshape(DIM, S)
    acc += np.asarray(inputs["b_proj"], np.float64)[:, None]
    out = np.ascontiguousarray(acc.T.astype(np.float32))
    return out, res


def kernel(**inputs) -> np.ndarray:
    out, _ = run(inputs, trace=False)
    return out


'''


# revision 7
# speedup vs baseline: 1.4589x; 1.2724x over previous
"""Trainium2 Bass kernel v2 for nn_Attention_72541997629647.

Sharding: segment x head-half. Core c = 2*si + hh owns segment si (768 rows,
the 4 segments are 128-aligned so no boundary masks) and heads
[8*hh, 8*hh+8). Each core computes qkv+RoPE for its 8 heads over its 768
rows, block-diagonal attention (which only needs rows inside the segment),
and a proj partial [DIM, 768] contracted over its 640 attention channels
(5 full 128-partition tiles -> no wasted contraction rows). The host sums
the two partials per segment and adds b_proj.

Attention channels are packed into 6 contraction tiles at a 96-row pitch
(16 zeroed pad rows per head) because engine AP partition offsets must be
32-aligned; the proj matmul then uses 6 mostly-full 128-partition
contraction tiles instead of 8 sparse 80-row ones.

vs v1 (heads-only sharding): per-core DMA drops 36.7MB -> ~11MB (x slice
instead of full x, bf16 I/O everywhere), proj PE cost drops 25.6us ->
19.2us, and emission is software-pipelined: A(g+1) qkv work interleaves
with B(g) attention so the PE never waits on the ACT exp chain, transposes
are deferred one tile, and the last head's tail is split so the final
serial norm->proj->DMA chain covers only 128 columns.

DMA discipline (the big hardware win): the HWDGE issue slot is a single
~650ns/DMA resource and transfers serialize on the DMA engines, so inputs
are few dense DMAs on one queue in first-use order; v-pad columns are
Pool-engine memsets instead of a broadcast DMA (which would generate 6144
descriptors and block the queue ~100us); outputs are dense per-chunk
buffers.

All matmuls run in bf16 (1 cycle/row at any size; fp32 psum accumulate).
The softmax denominator path stays f32/f32r. Falls back to the embedded
v1 kernel for non-canonical cu_seqlens.
"""

import os
import sys

for _p in ("/opt/trn_rl_repo", "/root/.axon_site/_ro/trn_rl_repo"):
    if os.path.isdir(_p) and _p not in sys.path:
        sys.path.insert(0, _p)

import numpy as np

import concourse.bacc as bacc
import concourse.bass as bass
import concourse.mybir as mybir
import concourse.tile as tile
from concourse.bass_utils import run_bass_kernel_spmd
from contextlib import ExitStack

S = 3072
DIM = 1280
H = 16
HD = 80
NCORES = 8
SEG = 768            # rows per segment
HPC = 8              # heads per core
NT = SEG // 128      # 6 s-tiles per core
GROUPS = 4           # head groups of 2 per core
VEXT = 97            # v cols: 80 v + 16 pad + ones at 96

F32 = mybir.dt.float32
F32R = mybir.dt.float32r
BF16 = mybir.dt.bfloat16
NPBF16 = mybir.dt.np(BF16)

CANON_SEGS = tuple((SEG * i, SEG * (i + 1)) for i in range(4))

_CACHE: dict = {}


def _segments_from_cu(cu_seqlens: np.ndarray) -> tuple:
    cu = np.asarray(cu_seqlens).astype(np.int64)
    seg = np.searchsorted(cu, np.arange(S), side="right") - 1
    change = np.nonzero(np.diff(seg))[0]
    starts = np.concatenate([[0], change + 1])
    ends = np.concatenate([change + 1, [S]])
    return tuple((int(a), int(b)) for a, b in zip(starts, ends))


def _chan_runs(h: int):
    """(src_row0, dst_ct, dst_row0, n) runs mapping head h's 80 attention
    channels into the 6x128 packed layout at 96-row pitch (16 pad rows per
    head). Runs are 32+32+16 so every src/dst partition base is 32-aligned
    (hardware requires engine AP partition offsets in {0,32,64,96})."""
    def legal(o):
        # engine AP partition rule: size<=32 at any 32-multiple, <=64 at
        # {0,64}, >64 only at 0
        if o == 0:
            return 128
        if o % 64 == 0:
            return 64
        if o % 32 == 0:
            return 32
        return 0
    runs, src = [], 0
    while src < HD:
        gpos = 96 * h + src
        o = gpos % 128
        n = min(legal(o), legal(src % 128), HD - src, 128 - o)
        assert n > 0
        runs.append((src, gpos // 128, o, n))
        src += n
    return runs


def _build(segments, loop_n: int = 1) -> "bacc.Bacc":
    assert segments == CANON_SEGS
    nc = bacc.Bacc("TRN2", target_bir_lowering=False, debug=False,
                   num_devices=NCORES)

    xblk_d = nc.dram_tensor("xblk", [NT, 128, 10, 128], BF16,
                            kind="ExternalInput")
    wqkvb_d = nc.dram_tensor("wqkvb", [GROUPS, 128, 10, 480], BF16,
                             kind="ExternalInput")
    bqkv_d = nc.dram_tensor("bqkv", [1, GROUPS, 480], BF16,
                            kind="ExternalInput")
    cosb_d = nc.dram_tensor("cosb", [128, NT, HD], F32, kind="ExternalInput")
    sinb_d = nc.dram_tensor("sinb", [128, NT, HD], F32, kind="ExternalInput")
    wptb_d = nc.dram_tensor("wptb", [128, 6, DIM], BF16, kind="ExternalInput")
    ident_d = nc.dram_tensor("ident", [128, 128], BF16, kind="ExternalInput")
    ones_d = nc.dram_tensor("onesrow", [1, 128], BF16, kind="ExternalInput")
    onesf_d = nc.dram_tensor("onesf", [1, 128], F32R, kind="ExternalInput")
    out0_d = nc.dram_tensor("outb0", [2, 128, 5, 512], BF16,
                            kind="ExternalOutput")
    out1a_d = nc.dram_tensor("outb1a", [2, 128, 5, 128], BF16,
                             kind="ExternalOutput")
    out1b_d = nc.dram_tensor("outb1b", [2, 128, 5, 128], BF16,
                             kind="ExternalOutput")

    with tile.TileContext(nc) as tc, ExitStack() as ctx:
        if loop_n > 1:
            ctx.enter_context(tc.For_i(0, loop_n, 1))
        per = ctx.enter_context(tc.tile_pool(name="persist", bufs=1))

        ident_sb = per.tile([128, 128], BF16, tag="ident")
        ones_sb = per.tile([1, 128], BF16, tag="ones")
        onesf_sb = per.tile([1, 128], F32R, tag="onesf")
        bqkv_sb = per.tile([1, GROUPS, 480], BF16, tag="bqkv")
        cos_sb = per.tile([128, NT, HD], F32, tag="cos")
        sin_sb = per.tile([128, NT, HD], F32, tag="sin")
        xt_sb = [per.tile([128, 10, 128], BF16, tag=f"xt{t}", name=f"xt{t}")
                 for t in range(NT)]
        wq_sb = [per.tile([128, 10, 480], BF16, tag=f"wq{g}", name=f"wq{g}")
                 for g in range(GROUPS)]
        wp_sb = per.tile([128, 6, DIM], BF16, tag="wp")
        qkTg = [per.tile([HD, 4, SEG], BF16, tag=f"qkT{g}", name=f"qkT{g}")
                for g in range(GROUPS)]
        v_sb = per.tile([128, HPC, NT, VEXT], BF16, tag="v")
        at_sb = per.tile([128, 6, SEG], BF16, tag="at")
        # the 16 pad rows per 96-row head granule are never written by the
        # normalize stage but ARE read by the proj matmul (against zero
        # weights) -- zero the tile so no inf/NaN garbage reaches the PE
        nc.gpsimd.memset(at_sb[:, :, :], 0.0)

        # v pad columns: zeros at 80:96, softmax-denominator ones at 96;
        # memset on the otherwise-idle Pool engine (a broadcast DMA here
        # would emit 6144 tiny descriptors and block the SP queue ~100us)
        nc.gpsimd.memset(v_sb[:, :, :, HD:VEXT - 1], 0.0)
        nc.gpsimd.memset(v_sb[:, :, :, VEXT - 1:VEXT], 1.0)

        # input DMAs: the HWDGE issue slot is a single shared ~625ns/DMA
        # resource, so use few dense DMAs, ordered by first use
        nc.sync.dma_start(out=xt_sb[0], in_=xblk_d[0])
        nc.sync.dma_start(out=wq_sb[0][:, 0:3, :], in_=wqkvb_d[0, :, 0:3, :])
        nc.sync.dma_start(out=wq_sb[0][:, 3:10, :], in_=wqkvb_d[0, :, 3:10, :])
        nc.sync.dma_start(out=xt_sb[1], in_=xblk_d[1])
        nc.sync.dma_start(out=ones_sb, in_=ones_d[:, :])
        nc.sync.dma_start(out=bqkv_sb, in_=bqkv_d[:, :, :])
        nc.sync.dma_start(out=xt_sb[2], in_=xblk_d[2])
        nc.sync.dma_start(out=cos_sb, in_=cosb_d[:, :, :])
        nc.sync.dma_start(out=sin_sb, in_=sinb_d[:, :, :])
        nc.sync.dma_start(out=xt_sb[3], in_=xblk_d[3])
        nc.sync.dma_start(out=ident_sb, in_=ident_d[:, :])
        nc.sync.dma_start(out=xt_sb[4], in_=xblk_d[4])
        nc.sync.dma_start(out=xt_sb[5], in_=xblk_d[5])
        nc.sync.dma_start(out=wq_sb[1], in_=wqkvb_d[1])
        nc.sync.dma_start(out=onesf_sb, in_=onesf_d[:, :])
        nc.sync.dma_start(out=wq_sb[2], in_=wqkvb_d[2])
        nc.sync.dma_start(out=wq_sb[3], in_=wqkvb_d[3])
        nc.sync.dma_start(out=wp_sb, in_=wptb_d[:, :, :])

        qpp = ctx.enter_context(tc.tile_pool(name="qpp", bufs=2, space="PSUM"))
        tpp = ctx.enter_context(tc.tile_pool(name="tpp", bufs=1, space="PSUM"))
        scp = ctx.enter_context(tc.tile_pool(name="scp", bufs=3, space="PSUM"))
        app = ctx.enter_context(tc.tile_pool(name="app", bufs=2, space="PSUM"))
        ropep = ctx.enter_context(tc.tile_pool(name="ropet", bufs=4))
        qkrop = ctx.enter_context(tc.tile_pool(name="qkro", bufs=4))
        expp = ctx.enter_context(tc.tile_pool(name="expp", bufs=12))
        smp = ctx.enter_context(tc.tile_pool(name="smalls", bufs=6))
        outp = ctx.enter_context(tc.tile_pool(name="outp", bufs=3))

        def emit_tp(g, t, ro):
            tp = tpp.tile([HD, 4, 128], BF16, tag="tp", name="tpps")
            for j in range(4):
                nc.tensor.transpose(tp[:, j, :], ro[:, HD * j:HD * (j + 1)],
                                    ident_sb)
            nc.vector.tensor_copy(qkTg[g][:, :, 128 * t:128 * (t + 1)], tp)

        def emit_A(g):
            """qkv + rope + v copy + qk transposes for head group g;
            yields once per s-tile so B work of the previous group can be
            interleaved between tiles.

            The transposes for tile t are deferred until after tile t+1's
            qkv matmuls so the PE never waits on the DVE RoPE chain."""
            pending = None
            for t in range(NT):
                qp = qpp.tile([128, 480], F32, tag="qp", name="qkvps")
                for dp in range(3):
                    nc.tensor.matmul(qp[:, :], lhsT=xt_sb[t][:, dp, :],
                                     rhs=wq_sb[g][:, dp, :],
                                     start=(dp == 0), stop=False)
                yield
                for dp in range(3, 10):
                    nc.tensor.matmul(qp[:, :], lhsT=xt_sb[t][:, dp, :],
                                     rhs=wq_sb[g][:, dp, :],
                                     start=False, stop=False)
                nc.tensor.matmul(qp[:, :], lhsT=ones_sb[:, :],
                                 rhs=bqkv_sb[:, g, :], start=False, stop=True)
                if pending is not None:
                    emit_tp(g, *pending)

                m1 = ropep.tile([128, 320], BF16, tag="m1")
                m2 = ropep.tile([128, 320], BF16, tag="m2")
                qk_h = qp[:, 0:320].rearrange("p (h d) -> p h d", h=4)
                cos_b = cos_sb[:, t:t + 1, :].to_broadcast([128, 4, HD])
                with nc.allow_low_precision("bf16 matmul inputs"):
                    nc.vector.tensor_mul(
                        m1.rearrange("p (h d) -> p h d", h=4), qk_h, cos_b)
                swap = qp[:, 0:320].rearrange(
                    "p (h x d) -> p h x d", h=4, x=2)[:, :, ::-1, :]
                sin_b = sin_sb[:, t:t + 1, :].rearrange(
                    "p t (x d) -> p (t x) d", x=2)[:, None, :, :] \
                    .to_broadcast([128, 4, 2, HD // 2])
                with nc.allow_low_precision("bf16 matmul inputs"):
                    nc.vector.tensor_mul(
                        m2.rearrange("p (h x d) -> p h x d", h=4, x=2),
                        swap, sin_b)
                ro = qkrop.tile([128, 320], BF16, tag="qkro")
                with nc.allow_low_precision("bf16 matmul inputs"):
                    nc.vector.tensor_add(ro, m1, m2)

                with nc.allow_low_precision("bf16 matmul inputs"):
                    nc.scalar.copy(
                        v_sb[:, 2 * g:2 * g + 2, t, 0:HD],
                        qp[:, 320:480].rearrange("p (e d) -> p e d", e=2))
                pending = (t, ro)
                yield
            emit_tp(g, *pending)

        def emit_B(h, qc0, qc1):
            """attention for core-local head h over q columns [qc0, qc1);
            yields between pipeline stages."""
            g, e = h // 2, h % 2
            qT = qkTg[g][:, e]
            kT = qkTg[g][:, 2 + e]
            qna = qc1 - qc0
            ap_ = app.tile([VEXT, 512], F32, tag="ap", name="attps")
            blocks = list(range(NT))
            for g0 in range(0, NT, 3):
                grp = blocks[g0:g0 + 3]
                exs = []
                for j in grp:
                    sc = scp.tile([128, 512], F32, tag="sc", name="scps")
                    nc.tensor.matmul(sc[:, :qna],
                                     lhsT=kT[:, 128 * j:128 * (j + 1)],
                                     rhs=qT[:, qc0:qc1],
                                     start=True, stop=True)
                    ex = expp.tile([128, 512], BF16, tag="expp")
                    nc.scalar.activation(ex[:, :qna], sc[:, :qna],
                                         mybir.ActivationFunctionType.Exp)
                    exs.append(ex)
                yield
                for j, ex in zip(grp, exs):
                    nc.tensor.matmul(ap_[:, :qna], lhsT=v_sb[:, h, j, :],
                                     rhs=ex[:, :qna],
                                     start=(j == 0), stop=(j == NT - 1))
                yield
            den = smp.tile([1, 512], F32R, tag="den", name="den")
            with nc.allow_low_precision("f32r matmul inputs"):
                if qna == 512:
                    nc.scalar.copy(den[:, :qna], ap_[96:97, :qna])
                else:
                    nc.vector.tensor_copy(den[:, :qna], ap_[96:97, :qna])
            yield
            bc = scp.tile([HD, 512], F32, tag="sc", name="bcps")
            nc.tensor.matmul(bc[:, :qna], lhsT=onesf_sb[:, 0:HD],
                             rhs=den[:, :qna], start=True, stop=True)
            rec = smp.tile([HD, 512], F32, tag="rec", name="rec")
            nc.vector.reciprocal(rec[:, :qna], bc[:, :qna])
            for (src, ct, dst, n) in _chan_runs(h):
                with nc.allow_low_precision("bf16 matmul inputs"):
                    nc.vector.tensor_mul(
                        at_sb[dst:dst + n, ct, qc0:qc1],
                        ap_[src:src + n, :qna],
                        rec[src:src + n, :qna])
            yield

        def emit_C(c0, c1, out_d, finer=False):
            """proj for s columns [c0, c1); dense output DMA per half-dim
            chunk (or per m-tile when finer), issue alternating SP/ACT."""
            n = c1 - c0
            for mh in range(2):
                ob = outp.tile([128, 5, n], BF16, tag="outp")
                for mm in range(5):
                    m = 5 * mh + mm
                    pp = qpp.tile([128, 512], F32, tag="qp", name="prps")
                    for ct in range(6):
                        nc.tensor.matmul(
                            pp[:, :n],
                            lhsT=wp_sb[:, ct, 128 * m:128 * (m + 1)],
                            rhs=at_sb[:, ct, c0:c1],
                            start=(ct == 0), stop=(ct == 5))
                    with nc.allow_low_precision("bf16 output"):
                        nc.scalar.copy(ob[:, mm, :], pp[:, :n])
                    if finer:
                        nc.sync.dma_start(
                            out=out_d[mh, :, mm, :], in_=ob[:, mm, :])
                    yield
                if not finer:
                    nc.sync.dma_start(out=out_d[mh], in_=ob)

        def chain(*gens):
            for gg in gens:
                yield from gg

        def zipgen(gen_a, gen_b):
            """Alternate single steps of two independent streams; drains
            both. Used for head pairs so one head's PE stages fill the
            other's exp-latency holes."""
            a_live, b_live = True, True
            while a_live or b_live:
                if a_live:
                    a_live = next(gen_a, StopIteration) is not StopIteration
                if b_live:
                    b_live = next(gen_b, StopIteration) is not StopIteration
                yield

        def interleave(gen_a, gen_b, ratio):
            """Alternate: one step of gen_a, then `ratio` steps of gen_b.
            Drains both."""
            a_live, b_live = True, True
            while a_live or b_live:
                if a_live:
                    a_live = next(gen_a, StopIteration) is not StopIteration
                if b_live:
                    for _ in range(ratio):
                        if next(gen_b, StopIteration) is StopIteration:
                            b_live = False
                            break

        def gen_group_B(g):
            for e in range(2):
                h = 2 * g + e
                yield from emit_B(h, 0, 512)
                yield from emit_B(h, 512, SEG)

        prev_b = None
        for g in range(GROUPS):
            if prev_b is None:
                for _ in emit_A(g):
                    pass
            else:
                interleave(emit_A(g), prev_b, 4)
            if g < GROUPS - 1:
                prev_b = gen_group_B(g)
        # tail: last group's B with C interleaved once its inputs are ready;
        # the last head's second chunk is split so the final serial
        # norm->proj->DMA chain covers only 128 columns
        h0, h1 = 2 * (GROUPS - 1), 2 * (GROUPS - 1) + 1
        for _ in zipgen(emit_B(h0, 0, 512), emit_B(h1, 0, 512)):
            pass
        interleave(chain(emit_B(h0, 512, SEG), emit_B(h1, 512, 640),
                         emit_B(h1, 640, SEG)),
                   emit_C(0, 512, out0_d), 3)
        for _ in emit_C(512, 640, out1a_d):
            pass
        for _ in emit_C(640, SEG, out1b_d):
            pass

    nc.compile()
    return nc


def _prep_inputs(x, cu_seqlens, rotary_pos_emb, w_qkv, b_qkv, w_proj, b_proj):
    """Host-side shard prep. Returns per-core input dicts."""
    scale = np.float32(1.0 / np.sqrt(np.float32(HD)))
    x = np.asarray(x, np.float32)
    w_qkv = np.asarray(w_qkv, np.float32)
    b_qkv = np.asarray(b_qkv, np.float32)
    w_proj = np.asarray(w_proj, np.float32)
    rot = np.asarray(rotary_pos_emb, np.float32)

    cosw = np.concatenate([np.cos(rot), np.cos(rot)], axis=1)
    sinw = np.concatenate([-np.sin(rot), np.sin(rot)], axis=1)

    ident = np.eye(128, dtype=NPBF16)
    onesrow = np.ones((1, 128), dtype=NPBF16)
    onesf = np.ones((1, 128), dtype=np.float32)

    in_maps = []
    for c in range(NCORES):
        si, hh = c // 2, c % 2
        s0 = SEG * si
        heads = list(range(8 * hh, 8 * hh + 8))

        xs = x[s0:s0 + SEG].astype(NPBF16)  # [768, 1280]
        # xblk[t, p, dp, s'] = x[s0+128t+s', 128dp+p]
        xblk = np.ascontiguousarray(
            xs.reshape(NT, 128, 10, 128).transpose(0, 3, 2, 1))

        # w_qkv rows in per-group order [q_a q_b k_a k_b v_a v_b] x 80
        idx = []
        for g in range(GROUPS):
            a, b = heads[2 * g], heads[2 * g + 1]
            for base, hsel in ((0, a), (0, b), (DIM, a), (DIM, b),
                               (2 * DIM, a), (2 * DIM, b)):
                idx.extend(range(base + hsel * HD, base + (hsel + 1) * HD))
        w_c = w_qkv[idx, :].copy()
        b_c = b_qkv[idx].copy()
        for g in range(GROUPS):
            w_c[480 * g:480 * g + 160] *= scale
            b_c[480 * g:480 * g + 160] *= scale
        # wqkvb[g, p, dp, cc] = w_c[480g+cc, 128dp+p] (dense per-group)
        wqkvb = np.ascontiguousarray(
            w_c.T.reshape(10, 128, GROUPS, 480).transpose(2, 1, 0, 3)
        ).astype(NPBF16)
        bqkvb = np.ascontiguousarray(b_c.reshape(1, GROUPS, 480)).astype(NPBF16)

        # rope tables [128, 6, 80] for this segment's rows
        cosb = np.ascontiguousarray(
            cosw[s0:s0 + SEG].reshape(NT, 128, HD).transpose(1, 0, 2))
        sinb = np.ascontiguousarray(
            sinw[s0:s0 + SEG].reshape(NT, 128, HD).transpose(1, 0, 2))

        # wptb[ct, p, m] = w_proj[m, chan(128ct+p)], chan c -> head
        # heads[c//80], dim c%80
        wptb = np.zeros((6, 128, DIM), np.float32)
        wv = wptb.reshape(768, DIM)
        for hl, habs in enumerate(heads):
            wv[96 * hl:96 * hl + HD] = w_proj[:, habs * HD:(habs + 1) * HD].T
        wptb = np.ascontiguousarray(
            wptb.transpose(1, 0, 2)).astype(NPBF16)  # [128, 6, DIM]

        in_maps.append({
            "xblk": xblk,
            "wqkvb": wqkvb,
            "bqkv": bqkvb,
            "cosb": cosb,
            "sinb": sinb,
            "wptb": np.ascontiguousarray(wptb),
            "ident": ident,
            "onesrow": onesrow,
            "onesf": onesf,
        })
    return in_maps


def run(inputs: dict, trace: bool = False):
    segments = _segments_from_cu(inputs["cu_seqlens"])
    if segments != CANON_SEGS:
        return _legacy_run(inputs, trace=trace)
    key = (segments, "v2")
    if key not in _CACHE:
        _CACHE[key] = _build(segments)
    nc = _CACHE[key]
    in_maps = _prep_inputs(
        inputs["x"], inputs["cu_seqlens"], inputs["rotary_pos_emb"],
        inputs["w_qkv"], inputs["b_qkv"], inputs["w_proj"], inputs["b_proj"])
    res = run_bass_kernel_spmd(nc, in_maps, core_ids=list(range(NCORES)),
                               trace=trace)
    acc = np.zeros((DIM, S), np.float64)
    for c, r in enumerate(res.results):
        si = c // 2
        part = np.zeros((128, 10, SEG), np.float64)
        for mh in range(2):
            part[:, 5 * mh:5 * (mh + 1), 0:512] = r["outb0"][mh]
            part[:, 5 * mh:5 * (mh + 1), 512:640] = r["outb1a"][mh]
            part[:, 5 * mh:5 * (mh + 1), 640:768] = r["outb1b"][mh]
        # partial[128m+p, s'] = part[p, m, s']
        acc[:, SEG * si:SEG * (si + 1)] += part.transpose(1, 0, 2).reshape(
            DIM, SEG)
    acc += np.asarray(inputs["b_proj"], np.float64)[:, None]
    out = np.ascontiguousarray(acc.T.astype(np.float32))
    return out, res


def kernel(**inputs) -> np.ndarray:
    out, _ = run(inputs, trace=False)
    return out


_LEGACY_MOD = None


def _legacy_run(inputs: dict, trace: bool = False):
    """General-segment fallback: the original heads-only-sharded kernel,
    embedded so kernel.py stays self-contained. Only used when cu_seqlens
    does not produce four 768-row segments."""
    global _LEGACY_MOD
    if _LEGACY_MOD is None:
        import types
        _LEGACY_MOD = types.ModuleType("kernel_legacy_embedded")
        exec(_LEGACY_SRC, _LEGACY_MOD.__dict__)
    return _LEGACY_MOD.run(inputs, trace=trace)


_LEGACY_SRC = r'''
"""Trainium2 Bass kernel for nn_Attention_72541997629647 (sparse varlen attention).

Computation (see problem reference):
  qkv = x @ w_qkv.T + b_qkv ; NeoX RoPE on q,k ; block-diagonal softmax
  attention from cu_seqlens segments ; out = (attn @ v) @ w_proj.T + b_proj

Sharding: tensor-parallel over heads. 16 heads / 8 cores = 2 heads per core.
Each core computes q/k/v for its 2 heads, runs block-diagonal attention, and
produces a partial projection output (full [DIM, S], transposed); the host
sums the 8 partials and adds b_proj, so the result is exact.

Device dataflow per core (all matmuls in float32r: full fp32 storage,
reduced-precision multiply at 4x the fp32 matmul rate):
  A) QKV: out_nat[s, 480] = xT-chunks.T @ w_chunks (+ bias via ones-row
     matmul); RoPE applied on the free dim (half-swap via negative-step AP,
     sign folded into the host-built sin table); q,k PE-transposed to
     [hd, S]; v kept natural with an appended ones column (denominator trick).
  B) per (head, segment, q-chunk): scoresT[k,q] = kT-block.T @ qT ; exp on
     ACT ; attn_extT[81, q] += v_ext.T @ exp accumulated over k-blocks; row 80
     is the softmax denominator. normalize = reciprocal + ones-matmul
     partition-broadcast + multiply.
  C) proj: outT[dim, s] += wpT-head.T @ attn_outT-head ; PSUM->SBUF copy on
     DVE; output written as fully-contiguous 1.25MB blocks (one dense
     descriptor chain per DMA, ~70us faster than 2KB-strided rows) and
     unscrambled on the host. b_proj is added host-side.
"""

import os
import sys

for _p in ("/opt/trn_rl_repo", "/root/.axon_site/_ro/trn_rl_repo"):
    if os.path.isdir(_p) and _p not in sys.path:
        sys.path.insert(0, _p)

import numpy as np

import concourse.bacc as bacc
import concourse.bass as bass
import concourse.mybir as mybir
import concourse.tile as tile
from concourse.bass_utils import run_bass_kernel_spmd
from contextlib import ExitStack

S = 3072
DIM = 1280
H = 16
HD = 80
NCORES = 8
HPC = H // NCORES          # heads per core = 2
QKDIM = 2 * HPC * HD       # 320 (q+k outdims per core)
ODIM = 3 * HPC * HD        # 480 (qkv outdims per core)
CDIM = HPC * HD            # 160 (attn channels per core)

F32 = mybir.dt.float32
F32R = mybir.dt.float32r
MM_DT = F32R               # matmul input dtype (F32R: 4x faster, ~1e-4 rel err)

_CACHE: dict = {}


def _segments_from_cu(cu_seqlens: np.ndarray) -> tuple:
    """Contiguous runs of equal segment id, exactly as the reference's
    searchsorted-based mask defines them."""
    cu = np.asarray(cu_seqlens).astype(np.int64)
    seg = np.searchsorted(cu, np.arange(S), side="right") - 1
    change = np.nonzero(np.diff(seg))[0]
    starts = np.concatenate([[0], change + 1])
    ends = np.concatenate([change + 1, [S]])
    return tuple((int(a), int(b)) for a, b in zip(starts, ends))


def _build(segments, loop_n: int = 1) -> "bacc.Bacc":
    nc = bacc.Bacc("TRN2", target_bir_lowering=False, debug=False,
                   num_devices=NCORES)

    xblk_d = nc.dram_tensor("xblk", [S // 512, 5, 128, 2, 512], MM_DT,
                        kind="ExternalInput")
    wqkvT_d = nc.dram_tensor("wqkvT", [DIM, ODIM], MM_DT, kind="ExternalInput")
    bqkv_d = nc.dram_tensor("bqkv", [1, ODIM], MM_DT, kind="ExternalInput")
    cosb_d = nc.dram_tensor("cosb", [S // 512, 128, 4, HD], F32,
                        kind="ExternalInput")
    sinb_d = nc.dram_tensor("sinb", [S // 512, 128, 4, HD], F32,
                        kind="ExternalInput")
    wpT_d = nc.dram_tensor("wpT", [CDIM, DIM], MM_DT, kind="ExternalInput")
    ident_d = nc.dram_tensor("ident", [128, 128], MM_DT, kind="ExternalInput")
    ones_d = nc.dram_tensor("onesrow", [1, 128], MM_DT, kind="ExternalInput")
    vpad_d = nc.dram_tensor("vpad", [17], MM_DT, kind="ExternalInput")
    # boundary-block 0/1 masks (segments not aligned to the 128 grid);
    # order must match the (head-agnostic) traversal below.
    bpairs = []
    for (s0, s1) in segments:
        for j in range(s0 // 128, -(-s1 // 128)):
            r0, r1 = max(0, s0 - 128 * j), min(128, s1 - 128 * j)
            if r0 > 0 or r1 < 128:
                bpairs.append((j, r0, r1))
    nbm = len(bpairs)
    bmask_d = (nc.dram_tensor("bmask", [nbm, 128], MM_DT, kind="ExternalInput")
               if nbm else None)
    outb_d = nc.dram_tensor("outb", [S // 512, 2, 128, 5, 512], F32,
                        kind="ExternalOutput")

    NT = S // 128   # 24 s-tiles
    NSS = S // 512  # 6 s-superchunks

    with tile.TileContext(nc) as tc, ExitStack() as ctx:
        if loop_n > 1:  # benchmarking only: repeat the whole body on-device
            ctx.enter_context(tc.For_i(0, loop_n, 1))
        per = ctx.enter_context(tc.tile_pool(name="persist", bufs=1))

        # small constants first so nothing cheap blocks the pipeline
        bqkv_sb = per.tile([1, ODIM], MM_DT, tag="bqkv")
        nc.sync.dma_start(out=bqkv_sb, in_=bqkv_d[:, :])
        ident_sb = per.tile([128, 128], MM_DT, tag="ident")
        nc.sync.dma_start(out=ident_sb, in_=ident_d[:, :])
        ones_sb = per.tile([1, 128], MM_DT, tag="ones")
        nc.sync.dma_start(out=ones_sb, in_=ones_d[:, :])
        # per-d-chunk qkv weights and per-superchunk rope tables: split so the
        # first matmul/rope can start after a fraction of the weight traffic
        wqkv_sb = [per.tile([128, ODIM], MM_DT, tag=f"wqkv{d}", name=f"wqkv{d}")
                   for d in range(10)]
        cos_sb = [per.tile([128, 4, HD], F32, tag=f"cos{ss}", name=f"cos{ss}")
                  for ss in range(NSS)]
        sin_sb = [per.tile([128, 4, HD], F32, tag=f"sin{ss}", name=f"sin{ss}")
                  for ss in range(NSS)]
        wp_sb = [per.tile([HD, DIM], MM_DT, tag=f"wp{h}", name=f"wp{h}") for h in range(HPC)]
        for h in range(HPC):
            nc.sync.dma_start(out=wp_sb[h], in_=wpT_d[h * HD:(h + 1) * HD, :])

        # v extended to 97 cols: 80 v-dims, 16 zero pad, ones col at 96 so the
        # denominator lands on a 32-aligned PSUM partition. Split per 512-s
        # superchunk so attention can start before all of phase A finishes.
        VEXT = 97
        v_sb = [[per.tile([128, 4, VEXT], MM_DT, tag=f"v{h}_{ss}",
                          name=f"v{h}_{ss}") for ss in range(NSS)]
                for h in range(HPC)]
        qkT = [[per.tile([HD, 512], MM_DT, tag=f"qkT{j}_{ss}",
                         name=f"qkT{j}_{ss}") for ss in range(NSS)]
               for j in range(2 * HPC)]
        att_o = [[per.tile([HD, 512], MM_DT, tag=f"atto{h}_{ss}",
                           name=f"atto{h}_{ss}") for ss in range(NSS)]
                 for h in range(HPC)]

        # one shared PSUM pool (8 bank-sized slots shared by every phase so
        # the scheduler can overlap A/B/C), plus top-level SBUF pools
        psp = ctx.enter_context(tc.tile_pool(name="ps", bufs=8, space="PSUM"))
        xtp = ctx.enter_context(tc.tile_pool(name="xt", bufs=6))
        ropep = ctx.enter_context(tc.tile_pool(name="ropet", bufs=2))
        qkrop = ctx.enter_context(tc.tile_pool(name="qkro", bufs=3))
        expp = ctx.enter_context(tc.tile_pool(name="expp", bufs=5))
        smp = ctx.enter_context(tc.tile_pool(name="smalls", bufs=2))
        outp = ctx.enter_context(tc.tile_pool(name="outp", bufs=2))

        if nbm:
            bmask_sb = per.tile([128, nbm], MM_DT, tag="bmask")
            nc.sync.dma_start(out=bmask_sb,
                              in_=bmask_d.ap().rearrange("n p -> p n"))
            bidx = {(j, r0, r1): i for i, (j, r0, r1) in enumerate(bpairs)}

        # ---------------- phase bodies (emitted interleaved below) --------
        def emit_A(ss):
            """QKV + RoPE + transposes for s-superchunk ss."""
            xts = []
            for dp in range(5):
                if ss == 0:
                    for d in (2 * dp, 2 * dp + 1):
                        nc.sync.dma_start(
                            out=wqkv_sb[d],
                            in_=wqkvT_d[128 * d:128 * (d + 1), :])
                xt = xtp.tile([128, 2, 512], MM_DT, tag="xt", name="xt")
                nc.sync.dma_start(out=xt, in_=xblk_d[ss, dp])
                xts.append(xt)
            nc.sync.dma_start(out=cos_sb[ss], in_=cosb_d[ss])
            nc.sync.dma_start(out=sin_sb[ss], in_=sinb_d[ss])
            for h in range(HPC):
                nc.sync.dma_start(
                    out=v_sb[h][ss][:, :, HD:VEXT],
                    in_=bass.AP(tensor=vpad_d, offset=0,
                                ap=[[0, 128], [0, 4], [1, VEXT - HD]]))
            tp_ps = [psp.tile([HD, 512], MM_DT, tag="ps", name="tpps")
                     for _ in range(2 * HPC)]
            nh = 2 * HPC  # 4 roped qk tensor-heads
            for sub in range(4):
                qp = psp.tile([128, ODIM], F32, tag="ps", name="qkvps")
                for d in range(10):
                    nc.tensor.matmul(
                        qp[:, :],
                        lhsT=xts[d // 2][:, d % 2, 128 * sub:128 * (sub + 1)],
                        rhs=wqkv_sb[d], start=(d == 0), stop=False)
                nc.tensor.matmul(qp[:, :], lhsT=ones_sb[:, :],
                                 rhs=bqkv_sb[:, :], start=False, stop=True)

                # RoPE over q,k: out = t*cos + halfswap(t)*sinsgn
                m1 = ropep.tile([128, QKDIM], F32, tag="m1")
                m2 = ropep.tile([128, QKDIM], F32, tag="m2")
                qk_h = qp[:, 0:QKDIM].rearrange("p (h d) -> p h d", h=nh)
                cos_b = cos_sb[ss][:, sub:sub + 1, :].to_broadcast(
                    [128, nh, HD])
                nc.vector.tensor_mul(
                    m1.rearrange("p (h d) -> p h d", h=nh), qk_h, cos_b)
                swap = qp[:, 0:QKDIM].rearrange(
                    "p (h x d) -> p h x d", h=nh, x=2)[:, :, ::-1, :]
                sin_b = sin_sb[ss][:, sub:sub + 1, :].rearrange(
                    "p t (x d) -> p (t x) d", x=2)[:, None, :, :] \
                    .to_broadcast([128, nh, 2, HD // 2])
                nc.vector.tensor_mul(
                    m2.rearrange("p (h x d) -> p h x d", h=nh, x=2),
                    swap, sin_b)
                ro = qkrop.tile([128, QKDIM], MM_DT, tag="qkro")
                with nc.allow_low_precision("f32r matmul inputs"):
                    nc.vector.tensor_add(ro, m1, m2)

                # v natural copy (its bias already in psum)
                for h in range(HPC):
                    nc.scalar.copy(
                        v_sb[h][ss][:, sub, 0:HD],
                        qp[:, QKDIM + HD * h:QKDIM + HD * (h + 1)])

                # transpose roped q,k to [hd, s]
                for j in range(2 * HPC):
                    nc.tensor.transpose(
                        tp_ps[j][:, 128 * sub:128 * (sub + 1)],
                        ro[:, HD * j:HD * (j + 1)], ident_sb)
            for j in range(2 * HPC):
                nc.scalar.copy(qkT[j][ss], tp_ps[j])

        def emit_B(seg):
            """block-diagonal attention for one segment (both heads)."""
            s0, s1 = seg
            jb0, jb1 = s0 // 128, -(-s1 // 128)
            # q-chunks aligned to the 512 grid so each lives in one tile
            g = (s0 // 512) * 512
            qchunks = []
            while g < s1:
                q0, q1 = max(s0, g), min(s1, g + 512)
                if q1 > q0:
                    qchunks.append((q0, q1))
                g += 512
            for q0, q1 in qchunks:
                qn = q1 - q0
                ss_q, c0 = q0 // 512, q0 % 512
                # fp32r matmuls need an even/4-aligned moving dim: widen the
                # compute window to 4-aligned columns (scratch cols unread)
                qa0 = q0 - (q0 % 4)
                qa1 = min(512 * (ss_q + 1), q1 + ((-q1) % 4))
                qna, off, ca0 = qa1 - qa0, q0 - qa0, qa0 % 512
                for h in range(HPC):
                    ap_ = psp.tile([VEXT, 512], F32, tag="ps", name="attps")
                    blocks = list(range(jb0, jb1))
                    for g0 in range(0, len(blocks), 4):
                        grp = blocks[g0:g0 + 4]
                        exs = []
                        for j in grp:
                            kTt = qkT[HPC + h][j // 4]
                            sc = psp.tile([128, 512], F32, tag="ps",
                                          name="scps")
                            nc.tensor.matmul(
                                sc[:, :qna],
                                lhsT=kTt[:, 128 * (j % 4):128 * (j % 4 + 1)],
                                rhs=qkT[h][ss_q][:, ca0:ca0 + qna],
                                start=True, stop=True)
                            ex = expp.tile([128, 512], MM_DT, tag="expp")
                            nc.scalar.activation(
                                ex[:, :qna], sc[:, :qna],
                                mybir.ActivationFunctionType.Exp)
                            r0, r1 = max(0, s0 - 128 * j), min(128, s1 - 128 * j)
                            if r0 > 0 or r1 < 128:
                                # zero out-of-segment rows of this block
                                mi = bidx[(j, r0, r1)]
                                with nc.allow_low_precision("f32r inputs"):
                                    nc.vector.tensor_mul(
                                        ex[:, :qna], ex[:, :qna],
                                        bmask_sb[:, mi:mi + 1]
                                        .to_broadcast([128, qna]))
                            exs.append(ex)
                        for j, ex in zip(grp, exs):
                            nc.tensor.matmul(
                                ap_[:, :qna],
                                lhsT=v_sb[h][j // 4][:, j % 4, :],
                                rhs=ex[:, :qna],
                                start=(j == blocks[0]),
                                stop=(j == blocks[-1]))
                    den = smp.tile([1, 512], MM_DT, tag="den", name="den")
                    with nc.allow_low_precision("f32r matmul inputs"):
                        nc.scalar.copy(den[:, :qna], ap_[96:97, :qna])
                    bc = psp.tile([HD, 512], F32, tag="ps", name="bcps")
                    nc.tensor.matmul(bc[:, :qna], lhsT=ones_sb[:, 0:HD],
                                     rhs=den[:, :qna], start=True, stop=True)
                    rec = smp.tile([HD, 512], F32, tag="rec", name="rec")
                    nc.vector.reciprocal(rec[:, :qna], bc[:, :qna])
                    with nc.allow_low_precision("f32r matmul inputs"):
                        nc.vector.tensor_mul(att_o[h][ss_q][:, c0:c0 + qn],
                                             ap_[0:HD, off:off + qn],
                                             rec[:, off:off + qn])

        def emit_C(sc_):
            """projection for output s-superchunk sc_.
            b_proj is added host-side after the cross-core partial sum."""
            for mh in range(2):
                ob = outp.tile([128, 5, 512], F32, tag="outp")
                for mm_ in range(5):
                    m = 5 * mh + mm_
                    pp = psp.tile([128, 512], F32, tag="ps", name="prps")
                    for h in range(HPC):
                        nc.tensor.matmul(
                            pp[:, :],
                            lhsT=wp_sb[h][:, 128 * m:128 * (m + 1)],
                            rhs=att_o[h][sc_],
                            start=(h == 0), stop=(h == HPC - 1))
                    nc.vector.tensor_copy(ob[:, mm_, :], pp)
                nc.sync.dma_start(out=outb_d[sc_, mh], in_=ob)

        # ---- interleaved driver: emit B as soon as its span is produced,
        # ---- C as soon as all segments covering its chunk are attended.
        segs_left = sorted(segments, key=lambda s: s[1])
        segs_done: list = []
        c_next = 0
        for ss in range(NSS):
            emit_A(ss)
            done_to = 512 * (ss + 1)
            while segs_left and segs_left[0][1] <= done_to:
                seg = segs_left.pop(0)
                emit_B(seg)
                segs_done.append(seg)
            covered = min((s0 for (s0, s1) in segs_left), default=S)
            while c_next < NSS and 512 * (c_next + 1) <= covered:
                emit_C(c_next)
                c_next += 1
        assert not segs_left
        while c_next < NSS:
            emit_C(c_next)
            c_next += 1

    nc.compile()
    return nc


def _prep_inputs(x, cu_seqlens, rotary_pos_emb, w_qkv, b_qkv, w_proj, b_proj):
    """Host-side shard prep. Returns per-core input dicts."""
    scale = np.float32(1.0 / np.sqrt(np.float32(HD)))
    xT = np.ascontiguousarray(np.asarray(x, np.float32).T)
    w_qkv = np.asarray(w_qkv, np.float32)
    b_qkv = np.asarray(b_qkv, np.float32)
    w_proj = np.asarray(w_proj, np.float32)
    b_proj = np.asarray(b_proj, np.float32)
    rot = np.asarray(rotary_pos_emb, np.float32)

    cosw = np.concatenate([np.cos(rot), np.cos(rot)], axis=1).astype(np.float32)
    sinw = np.concatenate([-np.sin(rot), np.sin(rot)], axis=1).astype(np.float32)
    # blocked layouts so every device DMA reads one dense contiguous region:
    # xblk[ss,dp,p,c,n] = xT[256dp+128c+p, 512ss+n]; cosb[ss,p,t,d] likewise
    xblk = np.ascontiguousarray(
        xT.reshape(5, 2, 128, 6, 512).transpose(3, 0, 2, 1, 4))
    cosb = np.ascontiguousarray(
        cosw.reshape(6, 4, 128, HD).transpose(0, 2, 1, 3))
    sinb = np.ascontiguousarray(
        sinw.reshape(6, 4, 128, HD).transpose(0, 2, 1, 3))
    ident = np.eye(128, dtype=np.float32)
    onesrow = np.ones((1, 128), dtype=np.float32)
    vpad = np.zeros(17, dtype=np.float32)
    vpad[16] = 1.0
    segments = _segments_from_cu(cu_seqlens)
    bmask_rows = []
    for (s0, s1) in segments:
        for j in range(s0 // 128, -(-s1 // 128)):
            r0, r1 = max(0, s0 - 128 * j), min(128, s1 - 128 * j)
            if r0 > 0 or r1 < 128:
                row = np.zeros(128, dtype=np.float32)
                row[r0:r1] = 1.0
                bmask_rows.append(row)
    bmask = np.stack(bmask_rows) if bmask_rows else None

    in_maps = []
    for c in range(NCORES):
        heads = [HPC * c + i for i in range(HPC)]
        idx = []
        for base in (0, DIM, 2 * DIM):           # q, k, v row blocks
            for h in heads:
                idx.extend(range(base + h * HD, base + (h + 1) * HD))
        w_c = w_qkv[idx, :].copy()
        b_c = b_qkv[idx].copy()
        w_c[:QKDIM // 2] *= scale                # scale q by 1/sqrt(HD)
        b_c[:QKDIM // 2] *= scale
        cdims = []
        for h in heads:
            cdims.extend(range(h * HD, (h + 1) * HD))
        wpT = np.ascontiguousarray(w_proj[:, cdims].T)  # [CDIM, DIM]
        in_maps.append({
            "xblk": xblk,
            "wqkvT": np.ascontiguousarray(w_c.T),
            "bqkv": np.ascontiguousarray(b_c[None, :]),
            "cosb": cosb,
            "sinb": sinb,
            "wpT": wpT,
            "ident": ident,
            "onesrow": onesrow,
            "vpad": vpad,
        })
        if bmask is not None:
            in_maps[-1]["bmask"] = bmask
    return in_maps


def run(inputs: dict, trace: bool = False):
    """Build (cached), run on 8 cores, return (out [S, DIM] fp32, results)."""
    segments = _segments_from_cu(inputs["cu_seqlens"])
    key = (segments, str(MM_DT))
    if key not in _CACHE:
        _CACHE[key] = _build(segments)
    nc = _CACHE[key]
    in_maps = _prep_inputs(
        inputs["x"], inputs["cu_seqlens"], inputs["rotary_pos_emb"],
        inputs["w_qkv"], inputs["b_qkv"], inputs["w_proj"], inputs["b_proj"])
    res = run_bass_kernel_spmd(nc, in_maps, core_ids=list(range(NCORES)),
                               trace=trace)
    acc = np.zeros((DIM, S), np.float64)
    for r in res.results:
        # blocked [sc, mh, p, c, n] -> [dim = 640*mh+128*c+p, s = 512*sc+n]
        acc += r["outb"].transpose(1, 3, 2, 0, 4).reshape(DIM, S)
    acc += np.asarray(inputs["b_proj"], np.float64)[:, None]
    out = np.ascontiguousarray(acc.T.astype(np.float32))
    return out, res


def kernel(**inputs) -> np.ndarray:
    out, _ = run(inputs, trace=False)
    return out


'''
